# revision 1
# baseline (speedup 1.0000x reference)
"""Associative-embedding (AE) loss on 8 TRN2 NeuronCores, data-parallel over batch.

Reference computation (per batch image b):
  g[m,k,:]   = tags[b, idx[b,m,k], :]                       (gather, T=8)
  mean[m,:]  = sum_k vf*g / max(cnt,1)                      (cnt = sum_k vf)
  pull       = (1/max(n,1)) * sum_m (1/max(cnt,1)) * sum_k vf * mean_t (g-mean)^2
  push       = 0.5/max(n(n-1),1) * sum_{i!=j valid} exp(-||mean_i-mean_j||^2)  (if n>1)
  out[b]     = [push, pull]

Sharding: batch dim B=64 split across 8 cores (8 images each). All reductions
are batch-local so no collectives are needed; the host concatenates per-core
outputs. Per core: two partition tiles of 4 images x 30 persons = 120
partitions, joints*tagdim (17*8=136) on the free dim.

The gather uses the Pool engine's indirect DMA; HW consumes ONE index per
partition per instruction, so each tile needs 17 gathers ([120,1] int32
offsets -> [120,8] floats each). Their ~1us/instruction descriptor-generation
serializes on the Pool engine and dominates the kernel (~35us of ~45us), so
everything else is arranged to hide under that window:
  - constants (identity / ones / block mask) arrive via HWDGE DMA,
  - per-image people counts n and all n-derived output factors are computed
    from the validity flags alone, before the gathers finish,
  - the per-joint masking is one small op per joint column, consuming each
    gather as it lands (also keeps every op at <= 1 foreign semaphore wait,
    a hard ISA limit outside PE),
  - the mean is transposed progressively: joints 0..15 transpose into PSUM
    early, the last joint accumulates via a second transpose matmul.
The exposed tail is: Gram G = meanT.T meanT (PE) -> ee = exp(2G - r_i) with
the row norm as the activation bias (never overflows since the exponent is
<= r_j <= max||mean||^2), U = ee.T W, s = ones.T (Wp*U) with exp(-r_j) folded
into Wp and -n accumulated by an extra matmul, then a single elementwise op
applies the precomputed push/pull factors and one DMA writes [1, 16] out.
The pull term uses sum vf*g^2 - cnt*||mean||^2 (one big multiply + reduce)
rather than explicit per-joint differences.
"""

import numpy as np

import concourse.bass as bass
import concourse.tile as tile
from concourse import bacc, mybir
from concourse.bass_utils import run_bass_kernel_spmd
from concourse.tile_rust import add_dep_helper

B, N, T = 64, 65536, 8
M, K = 30, 17
NCORES = 8
BL = B // NCORES  # images per core
TB = 4            # images per partition tile
NT = BL // TB     # partition tiles per core
P = TB * M        # 120 partitions per tile
F32 = mybir.dt.float32
I32 = mybir.dt.int32

ALU = mybir.AluOpType
AX = mybir.AxisListType


def build_nc():
    nc = bacc.Bacc("TRN2", target_bir_lowering=False, debug=False, num_devices=NCORES)
    tags_ext = nc.declare_dram_parameter("tags", [BL * N, T], F32, isOutput=False)
    gidx_ext = nc.declare_dram_parameter("gidx", [BL * M, K], I32, isOutput=False)
    vf_ext = nc.declare_dram_parameter("vf", [BL * M, K], F32, isOutput=False)
    # packed constants: cols 0..P-1 identity, P ones column, P+2..P+5 block mask
    cst_ext = nc.declare_dram_parameter("cst", [P, P + 6], F32, isOutput=False)
    out_ext = nc.declare_dram_parameter("out", [1, BL * 2], F32, isOutput=True)

    with tile.TileContext(nc) as tc:
        with (
            tc.tile_pool(name="sb", bufs=1) as sb,
            tc.tile_pool(name="ps", bufs=1, space="PSUM") as ps,
        ):
            # Constants come in via HWDGE DMA so the Pool engine does nothing
            # but the 34 gather DMAs. Each non-Pool engine gets one "warm"
            # instruction that waits on the const DMA, so later instructions
            # need at most one foreign-semaphore wait (the ISA structs for
            # PE LoadWeights / DVE ops have a single sync-wait slot).
            cst = sb.tile([P, P + 6], F32, tag="cst")
            nc.scalar.dma_start(cst[:], cst_ext[:])
            ident = cst[:, 0:P]
            ones_c = cst[:, P:P + 1]
            # DVE-local copy of the block mask (bmv[p, b] = 1 if p // M == b)
            bmv = sb.tile([P, TB], F32, tag="bmv")
            nc.vector.tensor_copy(out=bmv[:], in_=cst[:, P + 2:P + 6])
            # PE warm-up: observe the const DMA semaphore once
            warm_ps = ps.tile([1, 1], F32, tag="warm", bufs=1, space="PSUM")
            nc.tensor.matmul(out=warm_ps[:], lhsT=ones_c, rhs=ones_c, start=True, stop=True)
            out_sb = sb.tile([1, BL * 2], F32, tag="out_sb")
            stats_ps = ps.tile([1, NT * 8], F32, tag="stats", bufs=1, space="PSUM")
            n_ps = ps.tile([1, BL], F32, tag="nps", bufs=1, space="PSUM")
            _prev = None

            for t in range(NT):
                R = t * P
                gi = sb.tile([P, K], I32, tag=f"gi{t}")
                vf = sb.tile([P, K], F32, tag=f"vf{t}")
                nc.sync.dma_start(gi[:], gidx_ext[R:R + P, :])
                nc.scalar.dma_start(vf[:], vf_ext[R:R + P, :])
                vfv = sb.tile([P, K], F32, tag=f"vfv{t}")
                nc.vector.tensor_copy(out=vfv[:], in_=vf[:])

                # gather: g[p, k*T:(k+1)*T] = tags[gi[p, k], :]
                # (HW indirect DMA consumes one index per partition, so one
                #  transfer per joint column)
                g = sb.tile([P, K * T], F32, tag=f"g{t}")
                for k in range(K):
                    _ins = nc.gpsimd.indirect_dma_start(
                        out=g[:, k * T:(k + 1) * T],
                        out_offset=None,
                        in_=tags_ext[:],
                        in_offset=bass.IndirectOffsetOnAxis(ap=gi[:, k:k + 1], axis=0),
                    )
                    # order-only chain keeps the Pool engine's gather order
                    # tile-0-first so it never stalls on a late index load
                    if _prev is not None:
                        add_dep_helper(_ins.ins, _prev.ins, sync=False, reason="pool gather order")
                    _prev = _ins

                cnt = sb.tile([P, 1], F32, tag=f"cnt{t}")
                nc.vector.reduce_sum(out=cnt[:], in_=vfv[:], axis=AX.X)
                scnt = sb.tile([P, 1], F32, tag=f"scnt{t}")
                nc.vector.tensor_scalar_max(out=scnt[:], in0=cnt[:], scalar1=1.0)
                icnt = sb.tile([P, 1], F32, tag=f"icnt{t}")
                nc.vector.reciprocal(out=icnt[:], in_=scnt[:])
                h = sb.tile([P, 1], F32, tag=f"h{t}")
                nc.vector.tensor_scalar_min(out=h[:], in0=cnt[:], scalar1=1.0)

                # masked joints, one op per joint column so each waits on only its
                # own gather DMA's queue semaphore
                gm = sb.tile([P, K * T], F32, tag=f"gm{t}")
                gm_last = None
                for k in range(K):
                    gm_last = nc.vector.tensor_scalar_mul(
                        out=gm[:, k * T:(k + 1) * T], in0=g[:, k * T:(k + 1) * T],
                        scalar1=vfv[:, k:k + 1],
                    )

                # progressive mean: partial sum over joints 0..K-2 is ready one
                # gather earlier; the last joint's contribution is accumulated
                # into the transposed mean in PSUM by a second transpose matmul.
                mna = sb.tile([P, T], F32, tag=f"mna{t}")
                nc.vector.reduce_sum(
                    out=mna[:],
                    in_=gm[:, 0:(K - 1) * T].rearrange("p (k t) -> p t k", t=T),
                    axis=AX.X,
                )
                nc.vector.tensor_scalar_mul(out=mna[:], in0=mna[:], scalar1=icnt[:])
                # vic = vf[K-1]*icnt is ready early, so mnb fires straight off
                # the last gather's landing (parallel with its gm mask op)
                vic = sb.tile([P, 1], F32, tag=f"vic{t}")
                nc.vector.tensor_tensor(
                    out=vic[:], in0=vfv[:, K - 1:K], in1=icnt[:], op=ALU.mult,
                )
                mnb = sb.tile([P, T], F32, tag=f"mnb{t}")
                mnb_op = nc.vector.tensor_scalar_mul(
                    out=mnb[:], in0=g[:, (K - 1) * T:K * T], scalar1=vic[:],
                )
                # the last joint's mask op only feeds the slack pull path;
                # let the critical-path mnb run first on DVE
                add_dep_helper(gm_last.ins, mnb_op.ins, sync=False, reason="dve mnb first")
                tp = ps.tile([T, P], F32, tag="psA", bufs=3, space="PSUM")
                nc.tensor.matmul(out=tp[:], lhsT=mna[:], rhs=ident, is_transpose=True, start=True, stop=False)
                nc.tensor.matmul(out=tp[:], lhsT=mnb[:], rhs=ident, is_transpose=True, start=False, stop=True)
                meant = sb.tile([T, P], F32, tag=f"meant{t}")
                mcopy = nc.vector.tensor_copy(out=meant[:], in_=tp[:])
                d2p = ps.tile([P, P], F32, tag="psB", bufs=2, space="PSUM")
                nc.tensor.matmul(out=d2p[:], lhsT=meant[:], rhs=meant[:], start=True, stop=True)

                # full per-person mean (for rneg and the pull term)
                mn = sb.tile([P, T], F32, tag=f"mn{t}")
                nc.vector.tensor_tensor(out=mn[:], in0=mna[:], in1=mnb[:], op=ALU.add)
                # rneg = -||mean||^2 per person; er = exp(-r)
                msq = sb.tile([P, T], F32, tag=f"msq{t}")
                nc.vector.scalar_tensor_tensor(
                    out=msq[:], in0=mn[:], scalar=-1.0, in1=mn[:],
                    op0=ALU.mult, op1=ALU.mult,
                )
                rneg = sb.tile([P, 1], F32, tag=f"rneg{t}")
                nc.vector.reduce_sum(out=rneg[:], in_=msq[:], axis=AX.X)
                er = sb.tile([P, 1], F32, tag=f"er{t}")
                e_act = nc.scalar.activation(out=er[:], in_=rneg[:], func=mybir.ActivationFunctionType.Exp)

                # pull via sum_k vf*(g-m)^2 = sum_k vf*g^2 - cnt*||m||^2
                # (gm*g = vf*g^2 since vf is 0/1). Ordered after the meanT copy
                # so these big DVE ops stay off the transpose->Gram critical path.
                sg = sb.tile([P, K * T], F32, tag=f"sg{t}")
                sg_i = nc.vector.tensor_tensor(out=sg[:], in0=gm[:], in1=g[:], op=ALU.mult)
                add_dep_helper(sg_i.ins, mcopy.ins, sync=False, reason="dve order")
                sgr = sb.tile([P, 1], F32, tag=f"sgr{t}")
                nc.vector.reduce_sum(
                    out=sgr[:], in_=sg[:].rearrange("p (k t) -> p k t", t=T), axis=AX.XY,
                )
                # pull numerator = sgr + cnt*rneg; pp = numerator / (T*cnt)
                crn = sb.tile([P, 1], F32, tag=f"crn{t}")
                nc.vector.tensor_tensor(out=crn[:], in0=cnt[:], in1=rneg[:], op=ALU.mult)
                nc.vector.tensor_tensor(out=crn[:], in0=sgr[:], in1=crn[:], op=ALU.add)
                pp = sb.tile([P, 1], F32, tag=f"pp{t}")
                nc.vector.tensor_scalar(
                    out=pp[:], in0=crn[:], scalar1=icnt[:], scalar2=1.0 / T,
                    op0=ALU.mult, op1=ALU.mult,
                )

                # ee[i, j] = exp(2 G[i,j] - r_i); the missing exp(-r_j) factor is
                # folded into the masked weight vector below. Exponent <= r_j so
                # this never overflows.
                ee = sb.tile([P, P], F32, tag=f"ee{t}")
                ee_act = nc.scalar.activation(
                    out=ee[:], in_=d2p[:], func=mybir.ActivationFunctionType.Exp,
                    scale=2.0, bias=rneg[:],
                )
                # keep er before ee on ACT so ee needs only the PE wait
                add_dep_helper(ee_act.ins, e_act.ins, sync=False, reason="act order")

                # W = bm*h and the per-image people count n: both depend only on
                # vf, so they complete during the gather window
                wt = sb.tile([P, TB], F32, tag=f"wt{t}")
                nc.vector.tensor_scalar_mul(out=wt[:], in0=bmv[:], scalar1=h[:])
                wtn = sb.tile([P, TB], F32, tag=f"wtn{t}")
                nc.vector.tensor_scalar_mul(out=wtn[:], in0=wt[:], scalar1=-1.0)
                nc.tensor.matmul(
                    out=n_ps[0:1, TB * t:TB * (t + 1)], lhsT=ones_c[:], rhs=wt[:],
                    start=True, stop=True,
                )
                # Wp = W * exp(-r): restores the exp(-r_j) column factor
                srhs = sb.tile([P, 12], F32, tag=f"srhs{t}")
                nc.vector.tensor_scalar_mul(out=srhs[:, 8:12], in0=wt[:], scalar1=er[:])
                up = ps.tile([P, TB], F32, tag="psA", bufs=3, space="PSUM")
                nc.tensor.matmul(out=up[:], lhsT=ee[:], rhs=wt[:], start=True, stop=True)
                nc.vector.tensor_tensor(out=srhs[:, 0:4], in0=srhs[:, 8:12], in1=up[:], op=ALU.mult)
                nc.vector.tensor_scalar_mul(out=srhs[:, 4:8], in0=bmv[:], scalar1=pp[:])

                # per-image partition sums -> stats[0, 8t + (4q + b)], q in {s, pull};
                # a second accumulating matmul folds the -n subtraction into the
                # s columns so the final math needs one fewer dependent op
                # PH and -n sums don't depend on SU; schedule them first so only
                # one matmul remains after SU on the critical path
                ph_mm = nc.tensor.matmul(
                    out=stats_ps[0:1, 8 * t + 4:8 * (t + 1)], lhsT=ones_c[:], rhs=srhs[:, 4:8],
                    start=True, stop=True,
                )
                s_mm1 = nc.tensor.matmul(
                    out=stats_ps[0:1, 8 * t:8 * t + 4], lhsT=ones_c[:], rhs=wtn[:],
                    start=True, stop=False,
                )
                add_dep_helper(s_mm1.ins, ph_mm.ins, sync=False, reason="pe stats order")
                nc.tensor.matmul(
                    out=stats_ps[0:1, 8 * t:8 * t + 4], lhsT=ones_c[:], rhs=srhs[:, 0:4],
                    start=False, stop=True,
                )

            # n-derived factors: ready long before the last gather lands.
            # fac packs [push factor (0.5*mask/max(n(n-1),1)) | pull factor
            # (1/max(n,1))] so a single op finishes both outputs.
            fac = sb.tile([1, 2 * BL], F32, tag="fac")
            iq = fac[:, 0:BL]
            ipn = fac[:, BL:2 * BL]
            nsb = sb.tile([1, BL], F32, tag="nsb")
            nc.vector.tensor_copy(out=nsb[:], in_=n_ps[:])
            n1 = sb.tile([1, BL], F32, tag="n1")
            nc.vector.tensor_scalar_max(out=n1[:], in0=nsb[:], scalar1=1.0)
            nc.vector.reciprocal(out=ipn, in_=n1[:])
            nm1 = sb.tile([1, BL], F32, tag="nm1")
            nc.vector.tensor_scalar(out=nm1[:], in0=nsb[:], scalar1=1.0, scalar2=None, op0=ALU.subtract)
            q = sb.tile([1, BL], F32, tag="q")
            nc.vector.tensor_tensor(out=q[:], in0=nsb[:], in1=nm1[:], op=ALU.mult)
            nc.vector.tensor_scalar_max(out=q[:], in0=q[:], scalar1=1.0)
            nc.vector.reciprocal(out=iq, in_=q[:])
            # mask = clamp(n-1, 0, 1)  (n integer-valued: 1 iff n > 1); fold in 0.5
            nc.vector.tensor_scalar(out=nm1[:], in0=nm1[:], scalar1=0.0, scalar2=1.0, op0=ALU.max, op1=ALU.min)
            nc.vector.tensor_tensor(out=iq, in0=iq, in1=nm1[:], op=ALU.mult)
            nc.vector.tensor_scalar_mul(out=iq, in0=iq, scalar1=0.5)

            # final scalar math: ONE op depends on the last tile's stats
            # (s columns already hold s - n thanks to the accumulated -W sums)
            sv = stats_ps[:].rearrange("p (t q b) -> p q t b", t=NT, q=2, b=TB)
            ov = out_sb[:].rearrange("p (t b c) -> p c t b", t=NT, b=TB, c=2)
            fv = fac[:].rearrange("p (q t b) -> p q t b", q=2, t=NT, b=TB)
            nc.vector.tensor_tensor(out=ov, in0=sv, in1=fv, op=ALU.mult)

            nc.sync.dma_start(out_ext[:], out_sb[:])

    nc.compile()
    return nc


_NC_CACHE = {}


def _get_nc():
    if "nc" not in _NC_CACHE:
        _NC_CACHE["nc"] = build_nc()
    return _NC_CACHE["nc"]


def _make_consts():
    cst = np.zeros((P, P + 6), dtype=np.float32)
    cst[:, 0:P] = np.eye(P, dtype=np.float32)
    cst[:, P] = 1.0
    for b in range(TB):
        cst[b * M:(b + 1) * M, P + 2 + b] = 1.0
    return cst


def make_in_maps(tags, keypoints):
    tags = np.ascontiguousarray(np.asarray(tags, dtype=np.float32))
    kp = np.asarray(keypoints)
    # clip defensively: an out-of-range index would make the indirect DMA read
    # past the tags buffer and wedge the exec unit
    idx = np.clip(kp[..., 0].astype(np.int64), 0, N - 1)
    vf = (kp[..., 1] > 0).astype(np.float32)
    cst = _make_consts()
    in_maps = []
    for c in range(NCORES):
        sl = slice(BL * c, BL * (c + 1))
        tg = tags[sl].reshape(BL * N, T)
        fi = (np.arange(BL, dtype=np.int64)[:, None, None] * N + idx[sl]).astype(np.int32)
        in_maps.append({
            "tags": tg,
            "gidx": np.ascontiguousarray(fi.reshape(BL * M, K)),
            "vf": np.ascontiguousarray(vf[sl].reshape(BL * M, K)),
            "cst": cst,
        })
    return in_maps


def kernel(tags, keypoints):
    nc = _get_nc()
    in_maps = make_in_maps(tags, keypoints)
    last_err = None
    for _attempt in range(3):
        try:
            res = run_bass_kernel_spmd(nc, in_maps, core_ids=list(range(NCORES))).results
            break
        except Exception as e:  # a crashed predecessor can leave the NC wedged;
            last_err = e        # the failed attempt clears it, so retry
            import time
            time.sleep(1.0)
    else:
        raise last_err
    out = np.concatenate([res[c]["out"].reshape(BL, 2) for c in range(NCORES)], axis=0)
    return out.astype(np.float32)



# revision 6
# speedup vs baseline: 2.1061x; 2.1061x over previous
"""Associative-embedding (AE) loss on 8 TRN2 NeuronCores, data-parallel over batch.

Reference computation (per batch image b):
  g[m,k,:]   = tags[b, idx[b,m,k], :]                       (gather, T=8)
  mean[m,:]  = sum_k vf*g / max(cnt,1)                      (cnt = sum_k vf)
  pull       = (1/max(n,1)) * sum_m (1/max(cnt,1)) * sum_k vf * mean_t (g-mean)^2
  push       = 0.5/max(n(n-1),1) * sum_{i!=j valid} exp(-||mean_i-mean_j||^2)  (if n>1)
  out[b]     = [push, pull]

Sharding: batch dim B=64 split across 8 cores (8 images each); all reductions
are batch-local, no collectives; host concatenates per-core outputs.

Gather strategy: instead of 34 per-joint indirect DMAs (~1us of Pool SWDGE
descriptor-generation each), use TWO InstDMAGatherAnt instructions, one per
4-image quad. Each consumes int16 *block* indices and fetches the 256-byte
block (8 tag rows) containing each joint's row:
  - per-core tags are viewed as two [32768, 64] f32 halves (4 images each) so
    block indices fit int16's positive range,
  - item (slot k, partition p) of a gather lands at out[p, k, 0:64]; we place
    persons on partitions (120 of 128 used) and joints on slots,
  - the 8->1 sub-row selection is done on-chip with host-built masks
    M[p,k,s] = vf * (row & 7 == s), broadcast over the tag dim via a
    stride-0 AP, fused into one tensor_tensor_reduce per quad (gm = blk * M),
  - per-person sums then never need per-joint tensors: S1 = sum_{k,s} gm
    (DVE reduce keeping t), S2 = sum gm*blk = sum vf*g^2 (second ttr).
The remaining tail matches the old kernel: mean -> PE transpose -> Gram ->
exp(2G - r_i) with row-norm bias -> masked matmuls for push, and
pull = (S2 + cnt*rneg)/(T*cnt); n-derived factors come from the masks alone
and complete during the gather window.
"""

import numpy as np

import concourse.bass as bass
import concourse.tile as tile
from concourse import bacc, mybir
from concourse.bass_utils import run_bass_kernel_spmd
from concourse.tile_rust import add_dep_helper

B, N, T = 64, 65536, 8
M, K = 30, 17
NCORES = 8
BL = B // NCORES   # images per core
TB = 4             # images per quad
NQ = BL // TB      # quads per core (2)
P = 128            # partitions (TB*M = 120 used)
PU = TB * M        # used partitions
NI = K * P         # num_idxs per quad gather (2176)
NBLK = TB * N // 8  # 32768 blocks per tags half
F32 = mybir.dt.float32
I16 = mybir.dt.int16
U8 = mybir.dt.uint8

ALU = mybir.AluOpType
AX = mybir.AxisListType
ACT = mybir.ActivationFunctionType

# packed-constant byte layout (per partition) for the two input DMAs
#   pk1: [idxA (272B) | idxB (272B)]                      -> needed first
#   pk2: [M_A (544B) | M_B (544B) | ident (512B) | bmv(16B) | ones(4B)]
PK1_B = 544
PK2_B = 544 + 544 + 512 + 16 + 4


def build_nc():
    nc = bacc.Bacc("TRN2", target_bir_lowering=False, debug=False, num_devices=NCORES)
    tags_a = nc.declare_dram_parameter("tags_a", [NBLK, 64], F32, isOutput=False)
    tags_b = nc.declare_dram_parameter("tags_b", [NBLK, 64], F32, isOutput=False)
    pk1_ext = nc.declare_dram_parameter("pk1", [P, PK1_B], U8, isOutput=False)
    pk2_ext = nc.declare_dram_parameter("pk2", [P, PK2_B], U8, isOutput=False)
    out_ext = nc.declare_dram_parameter("out", [1, BL * 2], F32, isOutput=True)

    with tile.TileContext(nc) as tc:
        with (
            tc.tile_pool(name="sb", bufs=1) as sb,
            tc.tile_pool(name="ps", bufs=1, space="PSUM") as ps,
        ):
            pk1 = sb.tile([P, PK1_B], U8, tag="pk1")
            nc.sync.dma_start(pk1[:], pk1_ext[:])
            pk2 = sb.tile([P, PK2_B], U8, tag="pk2")
            nc.scalar.dma_start(pk2[:], pk2_ext[:])

            idx = [pk1[:, 0:272].bitcast(I16), pk1[:, 272:544].bitcast(I16)]
            msk = [
                pk2[:, 0:544].bitcast(F32).rearrange("p (k s) -> p k s", s=8),
                pk2[:, 544:1088].bitcast(F32).rearrange("p (k s) -> p k s", s=8),
            ]
            ident = pk2[:, 1088:1600].bitcast(F32)
            bmv = pk2[:, 1600:1616].bitcast(F32)
            ones_c = pk2[:, 1616:1620].bitcast(F32)

            srcs = [tags_a, tags_b]
            blk = [sb.tile([P, K, 64], F32, tag=f"blk{q}", name=f"blk{q}") for q in range(NQ)]
            gm = [sb.tile([P, K, 8, 8], F32, tag=f"gm{q}", name=f"gm{q}") for q in range(NQ)]
            sq = sb.tile([P, K * 64], F32, tag="sq")
            stats_ps = ps.tile([1, NQ * 8], F32, tag="stats", bufs=1, space="PSUM")
            n_ps = ps.tile([1, BL], F32, tag="nps", bufs=1, space="PSUM")
            out_sb = sb.tile([1, BL * 2], F32, tag="out_sb")

            # quad gathers, chunked: HW caps one dma_gather at 1024 indices,
            # so each quad is 3 instructions (slots 0-7 / 8-15 / 16)
            for q in range(NQ):
                for (k0, k1, ob) in ((0, 8, 0), (8, 16, 128), (16, 17, 256)):
                    ni = (k1 - k0) * P
                    nc.gpsimd.dma_gather(
                        out_ap=blk[q][:, k0:k1, :],
                        in_ap=srcs[q][:],
                        idxs_ap=pk1[:, 272 * q + ob:272 * q + ob + ni // 8].bitcast(I16),
                        num_idxs=ni,
                        num_idxs_reg=ni,
                        elem_size=64,
                    )

            # gather-window work: everything derivable from the masks alone
            cnt = sb.tile([P, NQ], F32, tag="cnt")
            icnt = sb.tile([P, NQ], F32, tag="icnt")
            h = sb.tile([P, NQ], F32, tag="h")
            wt = sb.tile([P, NQ * TB], F32, tag="wt")
            wtn = sb.tile([P, NQ * TB], F32, tag="wtn")
            for q in range(NQ):
                nc.vector.reduce_sum(
                    out=cnt[:, q:q + 1],
                    in_=msk[q].rearrange("p k s -> p (k s)"),
                    axis=AX.X,
                )
                nc.vector.tensor_scalar_max(out=icnt[:, q:q + 1], in0=cnt[:, q:q + 1], scalar1=1.0)
                nc.vector.reciprocal(out=icnt[:, q:q + 1], in_=icnt[:, q:q + 1])
                nc.vector.tensor_scalar_min(out=h[:, q:q + 1], in0=cnt[:, q:q + 1], scalar1=1.0)
                nc.vector.tensor_scalar_mul(
                    out=wt[:, TB * q:TB * (q + 1)], in0=bmv, scalar1=h[:, q:q + 1],
                )
                nc.tensor.matmul(
                    out=n_ps[0:1, TB * q:TB * (q + 1)], lhsT=ones_c, rhs=wt[:, TB * q:TB * (q + 1)],
                    start=True, stop=True,
                )
            nc.vector.tensor_scalar_mul(out=wtn[:], in0=wt[:], scalar1=-1.0)

            # n-derived output factors (ready long before gathers land):
            # fac packs [push factor | pull factor] per image
            fac = sb.tile([1, 2 * BL], F32, tag="fac")
            iq = fac[:, 0:BL]
            ipn = fac[:, BL:2 * BL]
            nsb = sb.tile([1, BL], F32, tag="nsb")
            nc.vector.tensor_copy(out=nsb[:], in_=n_ps[:])
            n1 = sb.tile([1, BL], F32, tag="n1")
            nc.vector.tensor_scalar_max(out=n1[:], in0=nsb[:], scalar1=1.0)
            nc.vector.reciprocal(out=ipn, in_=n1[:])
            nm1 = sb.tile([1, BL], F32, tag="nm1")
            nc.vector.tensor_scalar(out=nm1[:], in0=nsb[:], scalar1=1.0, scalar2=None, op0=ALU.subtract)
            qq = sb.tile([1, BL], F32, tag="qq")
            nc.vector.tensor_tensor(out=qq[:], in0=nsb[:], in1=nm1[:], op=ALU.mult)
            nc.vector.tensor_scalar_max(out=qq[:], in0=qq[:], scalar1=1.0)
            nc.vector.reciprocal(out=iq, in_=qq[:])
            nc.vector.tensor_scalar(out=nm1[:], in0=nm1[:], scalar1=0.0, scalar2=1.0, op0=ALU.max, op1=ALU.min)
            nc.vector.tensor_tensor(out=iq, in0=iq, in1=nm1[:], op=ALU.mult)
            nc.vector.tensor_scalar_mul(out=iq, in0=iq, scalar1=0.5)

            # per-quad main pipeline
            s2 = sb.tile([P, NQ], F32, tag="s2")
            s1 = [sb.tile([P, 8], F32, tag=f"s1{q}", name=f"s1{q}") for q in range(NQ)]
            mn = [sb.tile([P, 8], F32, tag=f"mn{q}", name=f"mn{q}") for q in range(NQ)]
            acc0 = sb.tile([P, 1], F32, tag="acc0")
            nc.vector.memset(acc0[:], 0.0)
            for q in range(NQ):
                mb = msk[q].broadcast_to((P, K, 8, 8))
                # gm = blk * M  (select + validity-mask in one op)
                nc.vector.tensor_tensor(
                    out=gm[q][:],
                    in0=blk[q][:].rearrange("p k (s t) -> p k s t", t=8),
                    in1=mb,
                    op=ALU.mult,
                )
                # S2 = sum vf*g^2 = sum gm^2 (ACT square-accumulate)
                nc.scalar.activation(
                    out=sq[:], in_=gm[q][:].rearrange("p k s t -> p (k s t)"),
                    func=ACT.Square, accum_out=s2[:, q:q + 1],
                )
                # S1[t] = sum_{k,s} gm
                nc.vector.reduce_sum(
                    out=s1[q][:],
                    in_=gm[q][:].rearrange("p k s t -> p t (k s)"),
                    axis=AX.X,
                )
                nc.vector.tensor_scalar_mul(out=mn[q][:], in0=s1[q][:], scalar1=icnt[:, q:q + 1])

                # rneg = -||mean||^2 ; er = exp(rneg)
                msq = sb.tile([P, 8], F32, tag=f"msq{q}")
                nc.vector.scalar_tensor_tensor(
                    out=msq[:], in0=mn[q][:], scalar=-1.0, in1=mn[q][:],
                    op0=ALU.mult, op1=ALU.mult,
                )
                rneg = sb.tile([P, 1], F32, tag=f"rneg{q}")
                nc.vector.reduce_sum(out=rneg[:], in_=msq[:], axis=AX.X)
                er = sb.tile([P, 1], F32, tag=f"er{q}")
                nc.scalar.activation(out=er[:], in_=rneg[:], func=ACT.Exp)

                # meanT via PE transpose, then Gram = meanT.T meanT
                tp = ps.tile([8, P], F32, tag="psA", bufs=2, space="PSUM")
                nc.tensor.matmul(out=tp[:], lhsT=mn[q][:], rhs=ident, is_transpose=True, start=True, stop=True)
                meant = sb.tile([8, P], F32, tag=f"meant{q}")
                nc.vector.tensor_copy(out=meant[:], in_=tp[:])
                d2p = ps.tile([P, P], F32, tag="psB", bufs=2, space="PSUM")
                nc.tensor.matmul(out=d2p[:], lhsT=meant[:], rhs=meant[:], start=True, stop=True)

                # ee[i,j] = exp(2G[i,j] - r_i); exp(-r_j) folded into wt below
                ee = sb.tile([P, P], F32, tag=f"ee{q}")
                nc.scalar.activation(out=ee[:], in_=d2p[:], func=ACT.Exp, scale=2.0, bias=rneg[:])

                # pull: pp = (S2 + cnt*rneg) * icnt / T
                crn = sb.tile([P, 1], F32, tag=f"crn{q}")
                nc.vector.tensor_tensor(out=crn[:], in0=cnt[:, q:q + 1], in1=rneg[:], op=ALU.mult)
                nc.vector.tensor_tensor(out=crn[:], in0=s2[:, q:q + 1], in1=crn[:], op=ALU.add)
                pp = sb.tile([P, 1], F32, tag=f"pp{q}")
                nc.vector.tensor_scalar(
                    out=pp[:], in0=crn[:], scalar1=icnt[:, q:q + 1], scalar2=1.0 / T,
                    op0=ALU.mult, op1=ALU.mult,
                )

                wtq = wt[:, TB * q:TB * (q + 1)]
                srhs = sb.tile([P, 12], F32, tag=f"srhs{q}")
                nc.vector.tensor_scalar_mul(out=srhs[:, 8:12], in0=wtq, scalar1=er[:])
                up = ps.tile([P, TB], F32, tag="psC", bufs=2, space="PSUM")
                nc.tensor.matmul(out=up[:], lhsT=ee[:], rhs=wtq, start=True, stop=True)
                nc.vector.tensor_tensor(out=srhs[:, 0:4], in0=srhs[:, 8:12], in1=up[:], op=ALU.mult)
                nc.vector.tensor_scalar_mul(out=srhs[:, 4:8], in0=bmv, scalar1=pp[:])

                # per-image sums; the -n correction accumulates into the s cols
                nc.tensor.matmul(
                    out=stats_ps[0:1, 8 * q + 4:8 * (q + 1)], lhsT=ones_c, rhs=srhs[:, 4:8],
                    start=True, stop=True,
                )
                nc.tensor.matmul(
                    out=stats_ps[0:1, 8 * q:8 * q + 4], lhsT=ones_c, rhs=wtn[:, TB * q:TB * (q + 1)],
                    start=True, stop=False,
                )
                nc.tensor.matmul(
                    out=stats_ps[0:1, 8 * q:8 * q + 4], lhsT=ones_c, rhs=srhs[:, 0:4],
                    start=False, stop=True,
                )

            # final: one op applies both factors, one DMA stores [1, 16]
            sv = stats_ps[:].rearrange("p (q c b) -> p c q b", q=NQ, c=2, b=TB)
            ov = out_sb[:].rearrange("p (q b c) -> p c q b", q=NQ, b=TB, c=2)
            fv = fac[:].rearrange("p (c q b) -> p c q b", c=2, q=NQ, b=TB)
            nc.vector.tensor_tensor(out=ov, in0=sv, in1=fv, op=ALU.mult)
            nc.sync.dma_start(out_ext[:], out_sb[:])

    nc.compile()
    return nc


_NC_CACHE = {}


def _get_nc():
    if "nc" not in _NC_CACHE:
        _NC_CACHE["nc"] = build_nc()
    return _NC_CACHE["nc"]


def _pack_consts(idx16, msks):
    """idx16: [NQ, 128, 136] int16 wrapped index layout; msks: [NQ, 128, K, 8] f32."""
    pk1 = np.zeros((P, PK1_B), dtype=np.uint8)
    pk1[:, 0:272] = idx16[0].view(np.uint8).reshape(P, 272)
    pk1[:, 272:544] = idx16[1].view(np.uint8).reshape(P, 272)
    pk2 = np.zeros((P, PK2_B), dtype=np.uint8)
    pk2[:, 0:544] = np.ascontiguousarray(msks[0].transpose(0, 1, 2)).view(np.uint8).reshape(P, 544)
    pk2[:, 544:1088] = np.ascontiguousarray(msks[1]).view(np.uint8).reshape(P, 544)
    ident = np.eye(P, dtype=np.float32)
    pk2[:, 1088:1600] = ident.view(np.uint8).reshape(P, 512)
    bmv = np.zeros((P, TB), dtype=np.float32)
    for b in range(TB):
        bmv[b * M:(b + 1) * M, b] = 1.0
    pk2[:, 1600:1616] = bmv.view(np.uint8).reshape(P, 16)
    ones = np.ones((P, 1), dtype=np.float32)
    pk2[:, 1616:1620] = ones.view(np.uint8).reshape(P, 4)
    return pk1, pk2


def make_in_maps(tags, keypoints):
    tags = np.asarray(tags, dtype=np.float32)
    kp = np.asarray(keypoints)
    idx = np.clip(kp[..., 0].astype(np.int64), 0, N - 1)   # [B, M, K]
    vf = (kp[..., 1] > 0).astype(np.float32)               # [B, M, K]

    in_maps = []
    for c in range(NCORES):
        halves = []
        idx16 = np.zeros((NQ, P, 136), dtype=np.int16)
        msks = np.zeros((NQ, P, K, 8), dtype=np.float32)
        for q in range(NQ):
            sl = slice(BL * c + TB * q, BL * c + TB * (q + 1))
            halves.append(np.ascontiguousarray(tags[sl].reshape(NBLK, 64)))
            iq_ = idx[sl]   # [TB, M, K]
            vq = vf[sl]
            # flat row within half -> block and sub-row
            rows = (np.arange(TB, dtype=np.int64)[:, None, None] * N + iq_)  # [TB, M, K]
            blk_q = (rows >> 3).astype(np.int16)      # [TB, M, K] in [0, 32768)
            sub_q = (rows & 7).astype(np.int64)
            # item (slot k, partition p): p = img*M + person; wrapped idx
            # layout per gather chunk (slots 0-7 / 8-15 / 16)
            pidx = np.arange(PU)
            img, per = pidx // M, pidx % M
            col = 0
            for (k0, k1) in ((0, 8), (8, 16), (16, 17)):
                ni = (k1 - k0) * P
                vals = np.zeros(ni, dtype=np.int16)
                for k in range(k0, k1):
                    vals[(k - k0) * P + pidx] = blk_q[img, per, k]
                wrapped = vals.reshape(ni // 16, 16).T   # [16, ni/16]
                idx16[q][:, col:col + ni // 16] = np.tile(wrapped, (8, 1))
                col += ni // 16
            # masks
            mq = np.zeros((P, K, 8), dtype=np.float32)
            mq[pidx[:, None], np.arange(K)[None, :], sub_q[img, per, :]] = vq[img, per, :]
            msks[q] = mq
        pk1, pk2 = _pack_consts(idx16, msks)
        in_maps.append({
            "tags_a": halves[0],
            "tags_b": halves[1],
            "pk1": pk1,
            "pk2": pk2,
        })
    return in_maps


def kernel(tags, keypoints):
    nc = _get_nc()
    in_maps = make_in_maps(tags, keypoints)
    last_err = None
    for _attempt in range(3):
        try:
            res = run_bass_kernel_spmd(nc, in_maps, core_ids=list(range(NCORES))).results
            break
        except Exception as e:  # a crashed predecessor can leave the NC wedged;
            last_err = e        # the failed attempt clears it, so retry
            import time
            time.sleep(1.0)
    else:
        raise last_err
    out = np.concatenate([res[c]["out"].reshape(BL, 2) for c in range(NCORES)], axis=0)
    return out.astype(np.float32)


# revision 8
# speedup vs baseline: 2.1320x; 1.0123x over previous
"""Associative-embedding (AE) loss on 8 TRN2 NeuronCores, data-parallel over batch.

Reference computation (per batch image b):
  g[m,k,:]   = tags[b, idx[b,m,k], :]                       (gather, T=8)
  mean[m,:]  = sum_k vf*g / max(cnt,1)                      (cnt = sum_k vf)
  pull       = (1/max(n,1)) * sum_m (1/max(cnt,1)) * sum_k vf * mean_t (g-mean)^2
  push       = 0.5/max(n(n-1),1) * sum_{i!=j valid} exp(-||mean_i-mean_j||^2)  (if n>1)
  out[b]     = [push, pull]

Sharding: batch dim B=64 split across 8 cores (8 images each); all reductions
are batch-local, no collectives; host concatenates per-core outputs.

Gather strategy: instead of 34 per-joint indirect DMAs (~1us of Pool SWDGE
descriptor-generation each), use TWO InstDMAGatherAnt instructions, one per
4-image quad. Each consumes int16 *block* indices and fetches the 256-byte
block (8 tag rows) containing each joint's row:
  - per-core tags are viewed as two [32768, 64] f32 halves (4 images each) so
    block indices fit int16's positive range,
  - item (slot k, partition p) of a gather lands at out[p, k, 0:64]; we place
    persons on partitions (120 of 128 used) and joints on slots,
  - the 8->1 sub-row selection is done on-chip with host-built masks
    M[p,k,s] = vf * (row & 7 == s), broadcast over the tag dim via a
    stride-0 AP, fused into one tensor_tensor_reduce per quad (gm = blk * M),
  - per-person sums then never need per-joint tensors: S1 = sum_{k,s} gm
    (DVE reduce keeping t), S2 = sum gm*blk = sum vf*g^2 (second ttr).
The remaining tail matches the old kernel: mean -> PE transpose -> Gram ->
exp(2G - r_i) with row-norm bias -> masked matmuls for push, and
pull = (S2 + cnt*rneg)/(T*cnt); n-derived factors come from the masks alone
and complete during the gather window.
"""

import numpy as np

import concourse.bass as bass
import concourse.tile as tile
from concourse import bacc, mybir
from concourse.bass_utils import run_bass_kernel_spmd
from concourse.tile_rust import add_dep_helper

B, N, T = 64, 65536, 8
M, K = 30, 17
NCORES = 8
BL = B // NCORES   # images per core
TB = 4             # images per quad
NQ = BL // TB      # quads per core (2)
P = 128            # partitions (TB*M = 120 used)
PU = TB * M        # used partitions
NI = K * P         # num_idxs per quad gather (2176)
NBLK = TB * N // 8  # 32768 blocks per tags half
F32 = mybir.dt.float32
I16 = mybir.dt.int16
U8 = mybir.dt.uint8

ALU = mybir.AluOpType
AX = mybir.AxisListType
ACT = mybir.ActivationFunctionType

# packed-constant byte layout (per partition) for the two input DMAs
#   pk1: [idxA (272B) | idxB (272B)]                      -> needed first
#   pk2: [M_A (544B) | M_B (544B) | ident (512B) | bmv(16B) | ones(4B)]
PK1_B = 544
PK2_B = 544 + 544 + 512 + 16 + 4


def build_nc():
    nc = bacc.Bacc("TRN2", target_bir_lowering=False, debug=False, num_devices=NCORES)
    tags_a = nc.declare_dram_parameter("tags_a", [NBLK, 64], F32, isOutput=False)
    tags_b = nc.declare_dram_parameter("tags_b", [NBLK, 64], F32, isOutput=False)
    pk1_ext = nc.declare_dram_parameter("pk1", [P, PK1_B], U8, isOutput=False)
    pk2_ext = nc.declare_dram_parameter("pk2", [P, PK2_B], U8, isOutput=False)
    out_ext = nc.declare_dram_parameter("out", [1, BL * 2], F32, isOutput=True)

    with tile.TileContext(nc) as tc:
        with (
            tc.tile_pool(name="sb", bufs=1) as sb,
            tc.tile_pool(name="ps", bufs=1, space="PSUM") as ps,
        ):
            pk1 = sb.tile([P, PK1_B], U8, tag="pk1")
            nc.sync.dma_start(pk1[:], pk1_ext[:])
            pk2 = sb.tile([P, PK2_B], U8, tag="pk2")
            nc.scalar.dma_start(pk2[:], pk2_ext[:])

            idx = [pk1[:, 0:272].bitcast(I16), pk1[:, 272:544].bitcast(I16)]
            msk = [
                pk2[:, 0:544].bitcast(F32).rearrange("p (k s) -> p k s", s=8),
                pk2[:, 544:1088].bitcast(F32).rearrange("p (k s) -> p k s", s=8),
            ]
            ident = pk2[:, 1088:1600].bitcast(F32)
            bmv = pk2[:, 1600:1616].bitcast(F32)
            ones_c = pk2[:, 1616:1620].bitcast(F32)

            srcs = [tags_a, tags_b]
            blk = [sb.tile([P, K, 64], F32, tag=f"blk{q}", name=f"blk{q}") for q in range(NQ)]
            gm = [sb.tile([P, K, 8, 8], F32, tag=f"gm{q}", name=f"gm{q}") for q in range(NQ)]
            sq = sb.tile([P, K * 64], F32, tag="sq")
            stats_ps = ps.tile([1, NQ * 8], F32, tag="stats", bufs=1, space="PSUM")
            n_ps = ps.tile([1, BL], F32, tag="nps", bufs=1, space="PSUM")
            out_sb = sb.tile([1, BL * 2], F32, tag="out_sb")

            # quad gathers, chunked: HW caps one dma_gather at 1024 indices,
            # so each quad is 3 instructions (slots 0-7 / 8-15 / 16).
            # Order A1 A2 B1 A3 B2 B3: quad A completes early (its whole tail
            # hides under quad B's transfers) and the bus stays near-saturated.
            CHUNKS = [(0, 0, 8), (0, 8, 16), (1, 0, 8), (0, 16, 17), (1, 8, 16), (1, 16, 17)]
            CHUNK_OFF = {(0, 8): 0, (8, 16): 128, (16, 17): 256}
            for (q, k0, k1) in CHUNKS:
                ob = 272 * q + CHUNK_OFF[(k0, k1)]
                ni = (k1 - k0) * P
                nc.gpsimd.dma_gather(
                    out_ap=blk[q][:, k0:k1, :],
                    in_ap=srcs[q][:],
                    idxs_ap=pk1[:, ob:ob + ni // 8].bitcast(I16),
                    num_idxs=ni,
                    num_idxs_reg=ni,
                    elem_size=64,
                )

            # gather-window work: everything derivable from the masks alone
            cnt = sb.tile([P, NQ], F32, tag="cnt")
            icnt = sb.tile([P, NQ], F32, tag="icnt")
            h = sb.tile([P, NQ], F32, tag="h")
            wt = sb.tile([P, NQ * TB], F32, tag="wt")
            wtn = sb.tile([P, NQ * TB], F32, tag="wtn")
            for q in range(NQ):
                nc.vector.reduce_sum(
                    out=cnt[:, q:q + 1],
                    in_=msk[q].rearrange("p k s -> p (k s)"),
                    axis=AX.X,
                )
                nc.vector.tensor_scalar_max(out=icnt[:, q:q + 1], in0=cnt[:, q:q + 1], scalar1=1.0)
                nc.vector.reciprocal(out=icnt[:, q:q + 1], in_=icnt[:, q:q + 1])
                nc.vector.tensor_scalar_min(out=h[:, q:q + 1], in0=cnt[:, q:q + 1], scalar1=1.0)
                nc.vector.tensor_scalar_mul(
                    out=wt[:, TB * q:TB * (q + 1)], in0=bmv, scalar1=h[:, q:q + 1],
                )
                nc.tensor.matmul(
                    out=n_ps[0:1, TB * q:TB * (q + 1)], lhsT=ones_c, rhs=wt[:, TB * q:TB * (q + 1)],
                    start=True, stop=True,
                )
            nc.vector.tensor_scalar_mul(out=wtn[:], in0=wt[:], scalar1=-1.0)

            # n-derived output factors (ready long before gathers land):
            # fac packs [push factor | pull factor] per image
            fac = sb.tile([1, 2 * BL], F32, tag="fac")
            iq = fac[:, 0:BL]
            ipn = fac[:, BL:2 * BL]
            nsb = sb.tile([1, BL], F32, tag="nsb")
            nc.vector.tensor_copy(out=nsb[:], in_=n_ps[:])
            n1 = sb.tile([1, BL], F32, tag="n1")
            nc.vector.tensor_scalar_max(out=n1[:], in0=nsb[:], scalar1=1.0)
            nc.vector.reciprocal(out=ipn, in_=n1[:])
            nm1 = sb.tile([1, BL], F32, tag="nm1")
            nc.vector.tensor_scalar(out=nm1[:], in0=nsb[:], scalar1=1.0, scalar2=None, op0=ALU.subtract)
            qq = sb.tile([1, BL], F32, tag="qq")
            nc.vector.tensor_tensor(out=qq[:], in0=nsb[:], in1=nm1[:], op=ALU.mult)
            nc.vector.tensor_scalar_max(out=qq[:], in0=qq[:], scalar1=1.0)
            nc.vector.reciprocal(out=iq, in_=qq[:])
            nc.vector.tensor_scalar(out=nm1[:], in0=nm1[:], scalar1=0.0, scalar2=1.0, op0=ALU.max, op1=ALU.min)
            nc.vector.tensor_tensor(out=iq, in0=iq, in1=nm1[:], op=ALU.mult)
            nc.vector.tensor_scalar_mul(out=iq, in0=iq, scalar1=0.5)

            # per-quad pipeline, processed per gather chunk in arrival order so
            # the DVE/ACT streams never head-of-line block on late data
            s2p = sb.tile([P, NQ * 3], F32, tag="s2p")
            s2 = sb.tile([P, NQ], F32, tag="s2")
            s1p = [sb.tile([P, 24], F32, tag=f"s1p{q}", name=f"s1p{q}") for q in range(NQ)]
            s1 = [sb.tile([P, 8], F32, tag=f"s1{q}", name=f"s1{q}") for q in range(NQ)]
            mn = [sb.tile([P, 8], F32, tag=f"mn{q}", name=f"mn{q}") for q in range(NQ)]

            def chunk_ops(q, k0, k1, j):
                nk = k1 - k0
                gm_s = gm[q][:, k0:k1]
                nc.vector.tensor_tensor(
                    out=gm_s,
                    in0=blk[q][:, k0:k1, :].rearrange("p k (s t) -> p k s t", t=8),
                    in1=msk[q][:, k0:k1].broadcast_to((P, nk, 8, 8)),
                    op=ALU.mult,
                )
                nc.vector.reduce_sum(
                    out=s1p[q][:, 8 * j:8 * (j + 1)],
                    in_=gm_s.rearrange("p k s t -> p t (k s)"),
                    axis=AX.X,
                )
                nc.scalar.activation(
                    out=sq[:, k0 * 64:k1 * 64],
                    in_=gm_s.rearrange("p k s t -> p (k s t)"),
                    func=ACT.Square, accum_out=s2p[:, 3 * q + j:3 * q + j + 1],
                )

            def finish_quad(q):
                sp = s1p[q]
                nc.vector.tensor_tensor(out=s1[q][:], in0=sp[:, 0:8], in1=sp[:, 8:16], op=ALU.add)
                nc.vector.tensor_tensor(out=s1[q][:], in0=s1[q][:], in1=sp[:, 16:24], op=ALU.add)
                nc.vector.reduce_sum(
                    out=s2[:, q:q + 1], in_=s2p[:, 3 * q:3 * q + 3], axis=AX.X,
                )
                nc.vector.tensor_scalar_mul(out=mn[q][:], in0=s1[q][:], scalar1=icnt[:, q:q + 1])

                # rneg = -||mean||^2 ; er = exp(rneg)
                msq = sb.tile([P, 8], F32, tag=f"msq{q}", name=f"msq{q}")
                nc.vector.scalar_tensor_tensor(
                    out=msq[:], in0=mn[q][:], scalar=-1.0, in1=mn[q][:],
                    op0=ALU.mult, op1=ALU.mult,
                )
                rneg = sb.tile([P, 1], F32, tag=f"rneg{q}", name=f"rneg{q}")
                nc.vector.reduce_sum(out=rneg[:], in_=msq[:], axis=AX.X)
                er = sb.tile([P, 1], F32, tag=f"er{q}", name=f"er{q}")
                nc.scalar.activation(out=er[:], in_=rneg[:], func=ACT.Exp)

                # meanT via PE transpose (copy to SBUF on ACT), Gram = meanT.T meanT
                tp = ps.tile([8, P], F32, tag="psA", bufs=2, space="PSUM", name=f"tp{q}")
                nc.tensor.matmul(out=tp[:], lhsT=mn[q][:], rhs=ident, is_transpose=True, start=True, stop=True)
                meant = sb.tile([8, P], F32, tag=f"meant{q}", name=f"meant{q}")
                nc.scalar.copy(out=meant[:], in_=tp[:])
                d2p = ps.tile([P, P], F32, tag="psB", bufs=2, space="PSUM", name=f"d2p{q}")
                nc.tensor.matmul(out=d2p[:], lhsT=meant[:], rhs=meant[:], start=True, stop=True)

                # ee[i,j] = exp(2G[i,j] - r_i); exp(-r_j) folded into wt below
                ee = sb.tile([P, P], F32, tag=f"ee{q}", name=f"ee{q}")
                nc.scalar.activation(out=ee[:], in_=d2p[:], func=ACT.Exp, scale=2.0, bias=rneg[:])

                # pull: pp = (S2 + cnt*rneg) * icnt / T
                crn = sb.tile([P, 1], F32, tag=f"crn{q}", name=f"crn{q}")
                nc.vector.tensor_tensor(out=crn[:], in0=cnt[:, q:q + 1], in1=rneg[:], op=ALU.mult)
                nc.vector.tensor_tensor(out=crn[:], in0=s2[:, q:q + 1], in1=crn[:], op=ALU.add)
                pp = sb.tile([P, 1], F32, tag=f"pp{q}", name=f"pp{q}")
                nc.vector.tensor_scalar(
                    out=pp[:], in0=crn[:], scalar1=icnt[:, q:q + 1], scalar2=1.0 / T,
                    op0=ALU.mult, op1=ALU.mult,
                )

                wtq = wt[:, TB * q:TB * (q + 1)]
                srhs = sb.tile([P, 12], F32, tag=f"srhs{q}", name=f"srhs{q}")
                nc.vector.tensor_scalar_mul(out=srhs[:, 8:12], in0=wtq, scalar1=er[:])
                up = ps.tile([P, TB], F32, tag="psC", bufs=2, space="PSUM", name=f"up{q}")
                nc.tensor.matmul(out=up[:], lhsT=ee[:], rhs=wtq, start=True, stop=True)
                nc.vector.tensor_tensor(out=srhs[:, 0:4], in0=srhs[:, 8:12], in1=up[:], op=ALU.mult)
                nc.vector.tensor_scalar_mul(out=srhs[:, 4:8], in0=bmv, scalar1=pp[:])

                # per-image sums; the -n correction accumulates into the s cols
                nc.tensor.matmul(
                    out=stats_ps[0:1, 8 * q + 4:8 * (q + 1)], lhsT=ones_c, rhs=srhs[:, 4:8],
                    start=True, stop=True,
                )
                nc.tensor.matmul(
                    out=stats_ps[0:1, 8 * q:8 * q + 4], lhsT=ones_c, rhs=wtn[:, TB * q:TB * (q + 1)],
                    start=True, stop=False,
                )
                nc.tensor.matmul(
                    out=stats_ps[0:1, 8 * q:8 * q + 4], lhsT=ones_c, rhs=srhs[:, 0:4],
                    start=False, stop=True,
                )

            jn = {0: 0, 1: 0}
            for (q, k0, k1) in CHUNKS:
                chunk_ops(q, k0, k1, jn[q])
                jn[q] += 1
                if jn[q] == 3:
                    finish_quad(q)

            # final: one op applies both factors, one DMA stores [1, 16]
            sv = stats_ps[:].rearrange("p (q c b) -> p c q b", q=NQ, c=2, b=TB)
            ov = out_sb[:].rearrange("p (q b c) -> p c q b", q=NQ, b=TB, c=2)
            fv = fac[:].rearrange("p (c q b) -> p c q b", c=2, q=NQ, b=TB)
            nc.vector.tensor_tensor(out=ov, in0=sv, in1=fv, op=ALU.mult)
            nc.sync.dma_start(out_ext[:], out_sb[:])

    nc.compile()
    return nc


_NC_CACHE = {}


def _get_nc():
    if "nc" not in _NC_CACHE:
        _NC_CACHE["nc"] = build_nc()
    return _NC_CACHE["nc"]


def _pack_consts(idx16, msks):
    """idx16: [NQ, 128, 136] int16 wrapped index layout; msks: [NQ, 128, K, 8] f32."""
    pk1 = np.zeros((P, PK1_B), dtype=np.uint8)
    pk1[:, 0:272] = idx16[0].view(np.uint8).reshape(P, 272)
    pk1[:, 272:544] = idx16[1].view(np.uint8).reshape(P, 272)
    pk2 = np.zeros((P, PK2_B), dtype=np.uint8)
    pk2[:, 0:544] = np.ascontiguousarray(msks[0].transpose(0, 1, 2)).view(np.uint8).reshape(P, 544)
    pk2[:, 544:1088] = np.ascontiguousarray(msks[1]).view(np.uint8).reshape(P, 544)
    ident = np.eye(P, dtype=np.float32)
    pk2[:, 1088:1600] = ident.view(np.uint8).reshape(P, 512)
    bmv = np.zeros((P, TB), dtype=np.float32)
    for b in range(TB):
        bmv[b * M:(b + 1) * M, b] = 1.0
    pk2[:, 1600:1616] = bmv.view(np.uint8).reshape(P, 16)
    ones = np.ones((P, 1), dtype=np.float32)
    pk2[:, 1616:1620] = ones.view(np.uint8).reshape(P, 4)
    return pk1, pk2


def make_in_maps(tags, keypoints):
    tags = np.asarray(tags, dtype=np.float32)
    kp = np.asarray(keypoints)
    idx = np.clip(kp[..., 0].astype(np.int64), 0, N - 1)   # [B, M, K]
    vf = (kp[..., 1] > 0).astype(np.float32)               # [B, M, K]

    in_maps = []
    for c in range(NCORES):
        halves = []
        idx16 = np.zeros((NQ, P, 136), dtype=np.int16)
        msks = np.zeros((NQ, P, K, 8), dtype=np.float32)
        for q in range(NQ):
            sl = slice(BL * c + TB * q, BL * c + TB * (q + 1))
            halves.append(np.ascontiguousarray(tags[sl].reshape(NBLK, 64)))
            iq_ = idx[sl]   # [TB, M, K]
            vq = vf[sl]
            # flat row within half -> block and sub-row
            rows = (np.arange(TB, dtype=np.int64)[:, None, None] * N + iq_)  # [TB, M, K]
            blk_q = (rows >> 3).astype(np.int16)      # [TB, M, K] in [0, 32768)
            sub_q = (rows & 7).astype(np.int64)
            # item (slot k, partition p): p = img*M + person; wrapped idx
            # layout per gather chunk (slots 0-7 / 8-15 / 16)
            pidx = np.arange(PU)
            img, per = pidx // M, pidx % M
            col = 0
            for (k0, k1) in ((0, 8), (8, 16), (16, 17)):
                ni = (k1 - k0) * P
                vals = np.zeros(ni, dtype=np.int16)
                for k in range(k0, k1):
                    vals[(k - k0) * P + pidx] = blk_q[img, per, k]
                wrapped = vals.reshape(ni // 16, 16).T   # [16, ni/16]
                idx16[q][:, col:col + ni // 16] = np.tile(wrapped, (8, 1))
                col += ni // 16
            # masks
            mq = np.zeros((P, K, 8), dtype=np.float32)
            mq[pidx[:, None], np.arange(K)[None, :], sub_q[img, per, :]] = vq[img, per, :]
            msks[q] = mq
        pk1, pk2 = _pack_consts(idx16, msks)
        in_maps.append({
            "tags_a": halves[0],
            "tags_b": halves[1],
            "pk1": pk1,
            "pk2": pk2,
        })
    return in_maps


def kernel(tags, keypoints):
    nc = _get_nc()
    in_maps = make_in_maps(tags, keypoints)
    last_err = None
    for _attempt in range(3):
        try:
            res = run_bass_kernel_spmd(nc, in_maps, core_ids=list(range(NCORES))).results
            break
        except Exception as e:  # a crashed predecessor can leave the NC wedged;
            last_err = e        # the failed attempt clears it, so retry
            import time
            time.sleep(1.0)
    else:
        raise last_err
    out = np.concatenate([res[c]["out"].reshape(BL, 2) for c in range(NCORES)], axis=0)
    return out.astype(np.float32)


# revision 11
# speedup vs baseline: 2.1803x; 1.0226x over previous
"""Associative-embedding (AE) loss on 8 TRN2 NeuronCores, data-parallel over batch.

Reference computation (per batch image b):
  g[m,k,:]   = tags[b, idx[b,m,k], :]                       (gather, T=8)
  mean[m,:]  = sum_k vf*g / max(cnt,1)                      (cnt = sum_k vf)
  pull       = (1/max(n,1)) * sum_m (1/max(cnt,1)) * sum_k vf * mean_t (g-mean)^2
  push       = 0.5/max(n(n-1),1) * sum_{i!=j valid} exp(-||mean_i-mean_j||^2)  (if n>1)
  out[b]     = [push, pull]

Sharding: batch dim B=64 split across 8 cores (8 images each); all reductions
are batch-local, no collectives; host concatenates per-core outputs.

Gather strategy: instead of 34 per-joint indirect DMAs (~1us of Pool SWDGE
descriptor-generation each), use TWO InstDMAGatherAnt instructions, one per
4-image quad. Each consumes int16 *block* indices and fetches the 256-byte
block (8 tag rows) containing each joint's row:
  - per-core tags are viewed as two [32768, 64] f32 halves (4 images each) so
    block indices fit int16's positive range,
  - item (slot k, partition p) of a gather lands at out[p, k, 0:64]; we place
    persons on partitions (120 of 128 used) and joints on slots,
  - the 8->1 sub-row selection is done on-chip with host-built masks
    M[p,k,s] = vf * (row & 7 == s), broadcast over the tag dim via a
    stride-0 AP, fused into one tensor_tensor_reduce per quad (gm = blk * M),
  - per-person sums then never need per-joint tensors: S1 = sum_{k,s} gm
    (DVE reduce keeping t), S2 = sum gm*blk = sum vf*g^2 (second ttr).
The remaining tail matches the old kernel: mean -> PE transpose -> Gram ->
exp(2G - r_i) with row-norm bias -> masked matmuls for push, and
pull = (S2 + cnt*rneg)/(T*cnt); n-derived factors come from the masks alone
and complete during the gather window.
"""

import numpy as np

import concourse.bass as bass
import concourse.tile as tile
from concourse import bacc, mybir
from concourse.bass_utils import run_bass_kernel_spmd
from concourse.tile_rust import add_dep_helper

B, N, T = 64, 65536, 8
M, K = 30, 17
NCORES = 8
BL = B // NCORES   # images per core
TB = 4             # images per quad
NQ = BL // TB      # quads per core (2)
P = 128            # partitions (TB*M = 120 used)
PU = TB * M        # used partitions
NI = K * P         # num_idxs per quad gather (2176)
NBLK = TB * N // 8  # 32768 blocks per tags half
F32 = mybir.dt.float32
I16 = mybir.dt.int16
U8 = mybir.dt.uint8

ALU = mybir.AluOpType
AX = mybir.AxisListType
ACT = mybir.ActivationFunctionType

# packed-constant byte layout (per partition) for the two input DMAs
#   pk1: [idxA (272B) | idxB (272B)]                      -> needed first
#   pk2: [M_A (544B) | M_B (544B) | ident (512B) | bmv(16B) | ones(4B) |
#         cnt(8B) | icnt(8B) | wt(32B) | wtn(32B) | fac(64B, partition 0)]
PK1_B = 544
PK2_B = 544 + 544 + 512 + 16 + 4 + 8 + 8 + 32 + 32 + 64


def build_nc():
    nc = bacc.Bacc("TRN2", target_bir_lowering=False, debug=False, num_devices=NCORES)
    tags_a = nc.declare_dram_parameter("tags_a", [NBLK, 64], F32, isOutput=False)
    tags_b = nc.declare_dram_parameter("tags_b", [NBLK, 64], F32, isOutput=False)
    pk1_ext = nc.declare_dram_parameter("pk1", [P, PK1_B], U8, isOutput=False)
    pk2_ext = nc.declare_dram_parameter("pk2", [P, PK2_B], U8, isOutput=False)
    out_ext = nc.declare_dram_parameter("out", [1, BL * 2], F32, isOutput=True)

    with tile.TileContext(nc) as tc:
        with (
            tc.tile_pool(name="sb", bufs=1) as sb,
            tc.tile_pool(name="ps", bufs=1, space="PSUM") as ps,
        ):
            pk1 = sb.tile([P, PK1_B], U8, tag="pk1")
            nc.sync.dma_start(pk1[:], pk1_ext[:])
            pk2 = sb.tile([P, PK2_B], U8, tag="pk2")
            nc.scalar.dma_start(pk2[:], pk2_ext[:])

            idx = [pk1[:, 0:272].bitcast(I16), pk1[:, 272:544].bitcast(I16)]
            msk = [
                pk2[:, 0:544].bitcast(F32).rearrange("p (k s) -> p k s", s=8),
                pk2[:, 544:1088].bitcast(F32).rearrange("p (k s) -> p k s", s=8),
            ]
            ident = pk2[:, 1088:1600].bitcast(F32)
            bmv = pk2[:, 1600:1616].bitcast(F32)
            ones_c = pk2[:, 1616:1620].bitcast(F32)
            cnt_q = [pk2[:, 1620 + 4 * q:1624 + 4 * q].bitcast(F32) for q in range(NQ)]
            icnt_q = [pk2[:, 1628 + 4 * q:1632 + 4 * q].bitcast(F32) for q in range(NQ)]
            wt_q = [pk2[:, 1636 + 16 * q:1652 + 16 * q].bitcast(F32) for q in range(NQ)]
            wtn_q = [pk2[:, 1668 + 16 * q:1684 + 16 * q].bitcast(F32) for q in range(NQ)]
            fac = pk2[0:1, 1700:1764].bitcast(F32)

            srcs = [tags_a, tags_b]
            blk = [sb.tile([P, K, 64], F32, tag=f"blk{q}", name=f"blk{q}") for q in range(NQ)]
            gm = [sb.tile([P, K, 8, 8], F32, tag=f"gm{q}", name=f"gm{q}") for q in range(NQ)]
            sq = sb.tile([P, K * 64], F32, tag="sq")
            stats_ps = ps.tile([1, NQ * 8], F32, tag="stats", bufs=1, space="PSUM")
            out_sb = sb.tile([1, BL * 2], F32, tag="out_sb")

            # quad gathers, chunked: HW caps one dma_gather at 1024 indices,
            # so each quad is 3 instructions (slots 0-7 / 8-15 / 16).
            # Order A1 A2 B1 A3 B2 B3: quad A completes early (its whole tail
            # hides under quad B's transfers) and the bus stays near-saturated.
            CHUNKS = [(0, 0, 8), (0, 8, 16), (1, 0, 8), (0, 16, 17), (1, 8, 16), (1, 16, 17)]
            CHUNK_OFF = {(0, 8): 0, (8, 16): 128, (16, 17): 256}
            for (q, k0, k1) in CHUNKS:
                ob = 272 * q + CHUNK_OFF[(k0, k1)]
                ni = (k1 - k0) * P
                nc.gpsimd.dma_gather(
                    out_ap=blk[q][:, k0:k1, :],
                    in_ap=srcs[q][:],
                    idxs_ap=pk1[:, ob:ob + ni // 8].bitcast(I16),
                    num_idxs=ni,
                    num_idxs_reg=ni,
                    elem_size=64,
                )

            # per-quad pipeline, processed per gather chunk in arrival order so
            # the DVE/ACT streams never head-of-line block on late data
            s2p = sb.tile([P, NQ * 3], F32, tag="s2p")
            s2 = sb.tile([P, NQ], F32, tag="s2")
            s1p = [sb.tile([P, 24], F32, tag=f"s1p{q}", name=f"s1p{q}") for q in range(NQ)]
            s1 = [sb.tile([P, 8], F32, tag=f"s1{q}", name=f"s1{q}") for q in range(NQ)]
            mn = [sb.tile([P, 8], F32, tag=f"mn{q}", name=f"mn{q}") for q in range(NQ)]

            def chunk_ops(q, k0, k1, j):
                nk = k1 - k0
                gm_s = gm[q][:, k0:k1]
                nc.vector.tensor_tensor(
                    out=gm_s,
                    in0=blk[q][:, k0:k1, :].rearrange("p k (s t) -> p k s t", t=8),
                    in1=msk[q][:, k0:k1].broadcast_to((P, nk, 8, 8)),
                    op=ALU.mult,
                )
                nc.vector.reduce_sum(
                    out=s1p[q][:, 8 * j:8 * (j + 1)],
                    in_=gm_s.rearrange("p k s t -> p t (k s)"),
                    axis=AX.X,
                )
                nc.scalar.activation(
                    out=sq[:, k0 * 64:k1 * 64],
                    in_=gm_s.rearrange("p k s t -> p (k s t)"),
                    func=ACT.Square, accum_out=s2p[:, 3 * q + j:3 * q + j + 1],
                )

            def finish_quad(q):
                sp = s1p[q]
                nc.vector.tensor_tensor(out=s1[q][:], in0=sp[:, 0:8], in1=sp[:, 8:16], op=ALU.add)
                nc.vector.tensor_tensor(out=s1[q][:], in0=s1[q][:], in1=sp[:, 16:24], op=ALU.add)
                nc.vector.reduce_sum(
                    out=s2[:, q:q + 1], in_=s2p[:, 3 * q:3 * q + 3], axis=AX.X,
                )
                nc.vector.tensor_scalar_mul(out=mn[q][:], in0=s1[q][:], scalar1=icnt_q[q])

                # rneg = -||mean||^2 ; er = exp(rneg)
                msq = sb.tile([P, 8], F32, tag=f"msq{q}", name=f"msq{q}")
                nc.vector.scalar_tensor_tensor(
                    out=msq[:], in0=mn[q][:], scalar=-1.0, in1=mn[q][:],
                    op0=ALU.mult, op1=ALU.mult,
                )
                rneg = sb.tile([P, 1], F32, tag=f"rneg{q}", name=f"rneg{q}")
                nc.vector.reduce_sum(out=rneg[:], in_=msq[:], axis=AX.X)
                er = sb.tile([P, 1], F32, tag=f"er{q}", name=f"er{q}")
                nc.scalar.activation(out=er[:], in_=rneg[:], func=ACT.Exp)

                # meanT via PE transpose (copy to SBUF on ACT), Gram = meanT.T meanT
                tp = ps.tile([8, P], F32, tag="psA", bufs=2, space="PSUM", name=f"tp{q}")
                nc.tensor.matmul(out=tp[:], lhsT=mn[q][:], rhs=ident, is_transpose=True, start=True, stop=True)
                meant = sb.tile([8, P], F32, tag=f"meant{q}", name=f"meant{q}")
                nc.scalar.copy(out=meant[:], in_=tp[:])
                d2p = ps.tile([P, P], F32, tag="psB", bufs=2, space="PSUM", name=f"d2p{q}")
                nc.tensor.matmul(out=d2p[:], lhsT=meant[:], rhs=meant[:], start=True, stop=True)

                # ee[i,j] = exp(2G[i,j] - r_i); exp(-r_j) folded into wt below
                ee = sb.tile([P, P], F32, tag=f"ee{q}", name=f"ee{q}")
                nc.scalar.activation(out=ee[:], in_=d2p[:], func=ACT.Exp, scale=2.0, bias=rneg[:])

                # pull: pp = (S2 + cnt*rneg) * icnt / T
                crn = sb.tile([P, 1], F32, tag=f"crn{q}", name=f"crn{q}")
                nc.vector.tensor_tensor(out=crn[:], in0=cnt_q[q], in1=rneg[:], op=ALU.mult)
                nc.vector.tensor_tensor(out=crn[:], in0=s2[:, q:q + 1], in1=crn[:], op=ALU.add)
                pp = sb.tile([P, 1], F32, tag=f"pp{q}", name=f"pp{q}")
                nc.vector.tensor_scalar(
                    out=pp[:], in0=crn[:], scalar1=icnt_q[q], scalar2=1.0 / T,
                    op0=ALU.mult, op1=ALU.mult,
                )

                wtq = wt_q[q]
                srhs = sb.tile([P, 12], F32, tag=f"srhs{q}", name=f"srhs{q}")
                nc.vector.tensor_scalar_mul(out=srhs[:, 8:12], in0=wtq, scalar1=er[:])
                up = ps.tile([P, TB], F32, tag="psC", bufs=2, space="PSUM", name=f"up{q}")
                nc.tensor.matmul(out=up[:], lhsT=ee[:], rhs=wtq, start=True, stop=True)
                nc.vector.tensor_tensor(out=srhs[:, 0:4], in0=srhs[:, 8:12], in1=up[:], op=ALU.mult)
                nc.vector.tensor_scalar_mul(out=srhs[:, 4:8], in0=bmv, scalar1=pp[:])

                # per-image sums; the -n correction accumulates into the s cols
                nc.tensor.matmul(
                    out=stats_ps[0:1, 8 * q + 4:8 * (q + 1)], lhsT=ones_c, rhs=srhs[:, 4:8],
                    start=True, stop=True,
                )
                nc.tensor.matmul(
                    out=stats_ps[0:1, 8 * q:8 * q + 4], lhsT=ones_c, rhs=wtn_q[q],
                    start=True, stop=False,
                )
                nc.tensor.matmul(
                    out=stats_ps[0:1, 8 * q:8 * q + 4], lhsT=ones_c, rhs=srhs[:, 0:4],
                    start=False, stop=True,
                )

            jn = {0: 0, 1: 0}
            for (q, k0, k1) in CHUNKS:
                chunk_ops(q, k0, k1, jn[q])
                jn[q] += 1
                if jn[q] == 3:
                    finish_quad(q)

            # final: one op applies both factors, one DMA stores [1, 16]
            sv = stats_ps[:].rearrange("p (q c b) -> p c q b", q=NQ, c=2, b=TB)
            ov = out_sb[:].rearrange("p (q b c) -> p c q b", q=NQ, b=TB, c=2)
            fv = fac.rearrange("p (c q b) -> p c q b", c=2, q=NQ, b=TB)
            nc.vector.tensor_tensor(out=ov, in0=sv, in1=fv, op=ALU.mult)
            nc.sync.dma_start(out_ext[:], out_sb[:])

    nc.compile()
    return nc


_NC_CACHE = {}


def _get_nc():
    if "nc" not in _NC_CACHE:
        _NC_CACHE["nc"] = build_nc()
    return _NC_CACHE["nc"]


def _pack_consts(idx16, msks, vfq):
    """idx16: [NQ,128,136] i16 wrapped; msks: [NQ,128,K,8] f32; vfq: [NQ,TB,M,K]."""
    pk1 = np.zeros((P, PK1_B), dtype=np.uint8)
    pk1[:, 0:272] = idx16[0].view(np.uint8).reshape(P, 272)
    pk1[:, 272:544] = idx16[1].view(np.uint8).reshape(P, 272)
    pk2 = np.zeros((P, PK2_B), dtype=np.uint8)
    pk2[:, 0:544] = np.ascontiguousarray(msks[0]).view(np.uint8).reshape(P, 544)
    pk2[:, 544:1088] = np.ascontiguousarray(msks[1]).view(np.uint8).reshape(P, 544)
    ident = np.eye(P, dtype=np.float32)
    pk2[:, 1088:1600] = ident.view(np.uint8).reshape(P, 512)
    bmv = np.zeros((P, TB), dtype=np.float32)
    for b in range(TB):
        bmv[b * M:(b + 1) * M, b] = 1.0
    pk2[:, 1600:1616] = bmv.view(np.uint8).reshape(P, 16)
    ones = np.ones((P, 1), dtype=np.float32)
    pk2[:, 1616:1620] = ones.view(np.uint8).reshape(P, 4)
    # per-person / per-image normalizers (keypoint metadata only)
    cnt = np.zeros((P, NQ), dtype=np.float32)
    fac = np.zeros(2 * BL, dtype=np.float32)
    wt = np.zeros((P, NQ * TB), dtype=np.float32)
    for q in range(NQ):
        cq = vfq[q].sum(axis=2).reshape(PU)          # [120]
        cnt[:PU, q] = cq
        h = np.minimum(cq, 1.0)
        wt[:PU, TB * q:TB * (q + 1)] = bmv[:PU] * h[:, None]
        n = h.reshape(TB, M).sum(axis=1)             # [4]
        iq = 0.5 * np.clip(n - 1.0, 0.0, 1.0) / np.maximum(n * (n - 1.0), 1.0)
        ipn = 1.0 / np.maximum(n, 1.0)
        fac[0 * BL + TB * q:0 * BL + TB * (q + 1)] = iq
        fac[1 * BL + TB * q:1 * BL + TB * (q + 1)] = ipn
    icnt = 1.0 / np.maximum(cnt, 1.0)
    pk2[:, 1620:1628] = cnt.view(np.uint8).reshape(P, 8)
    pk2[:, 1628:1636] = icnt.view(np.uint8).reshape(P, 8)
    pk2[:, 1636:1668] = wt.view(np.uint8).reshape(P, 32)
    pk2[:, 1668:1700] = (-wt).view(np.uint8).reshape(P, 32)
    pk2[0, 1700:1764] = fac.view(np.uint8)
    return pk1, pk2


def make_in_maps(tags, keypoints):
    tags = np.asarray(tags, dtype=np.float32)
    kp = np.asarray(keypoints)
    idx = np.clip(kp[..., 0].astype(np.int64), 0, N - 1)   # [B, M, K]
    vf = (kp[..., 1] > 0).astype(np.float32)               # [B, M, K]

    in_maps = []
    for c in range(NCORES):
        halves = []
        idx16 = np.zeros((NQ, P, 136), dtype=np.int16)
        msks = np.zeros((NQ, P, K, 8), dtype=np.float32)
        vfq = np.zeros((NQ, TB, M, K), dtype=np.float32)
        for q in range(NQ):
            sl = slice(BL * c + TB * q, BL * c + TB * (q + 1))
            halves.append(np.ascontiguousarray(tags[sl].reshape(NBLK, 64)))
            iq_ = idx[sl]   # [TB, M, K]
            vq = vf[sl]
            vfq[q] = vq
            # flat row within half -> block and sub-row
            rows = (np.arange(TB, dtype=np.int64)[:, None, None] * N + iq_)  # [TB, M, K]
            blk_q = (rows >> 3).astype(np.int16)      # [TB, M, K] in [0, 32768)
            sub_q = (rows & 7).astype(np.int64)
            # item (slot k, partition p): p = img*M + person; wrapped idx
            # layout per gather chunk (slots 0-7 / 8-15 / 16)
            pidx = np.arange(PU)
            img, per = pidx // M, pidx % M
            col = 0
            for (k0, k1) in ((0, 8), (8, 16), (16, 17)):
                ni = (k1 - k0) * P
                vals = np.zeros(ni, dtype=np.int16)
                for k in range(k0, k1):
                    vals[(k - k0) * P + pidx] = blk_q[img, per, k]
                wrapped = vals.reshape(ni // 16, 16).T   # [16, ni/16]
                idx16[q][:, col:col + ni // 16] = np.tile(wrapped, (8, 1))
                col += ni // 16
            # masks
            mq = np.zeros((P, K, 8), dtype=np.float32)
            mq[pidx[:, None], np.arange(K)[None, :], sub_q[img, per, :]] = vq[img, per, :]
            msks[q] = mq
        pk1, pk2 = _pack_consts(idx16, msks, vfq)
        in_maps.append({
            "tags_a": halves[0],
            "tags_b": halves[1],
            "pk1": pk1,
            "pk2": pk2,
        })
    return in_maps


def kernel(tags, keypoints):
    nc = _get_nc()
    in_maps = make_in_maps(tags, keypoints)
    last_err = None
    for _attempt in range(3):
        try:
            res = run_bass_kernel_spmd(nc, in_maps, core_ids=list(range(NCORES))).results
            break
        except Exception as e:  # a crashed predecessor can leave the NC wedged;
            last_err = e        # the failed attempt clears it, so retry
            import time
            time.sleep(1.0)
    else:
        raise last_err
    out = np.concatenate([res[c]["out"].reshape(BL, 2) for c in range(NCORES)], axis=0)
    return out.astype(np.float32)


# revision 14
# speedup vs baseline: 2.1858x; 1.0026x over previous
"""Associative-embedding (AE) loss on 8 TRN2 NeuronCores, data-parallel over batch.

Reference computation (per batch image b):
  g[m,k,:]   = tags[b, idx[b,m,k], :]                       (gather, T=8)
  mean[m,:]  = sum_k vf*g / max(cnt,1)                      (cnt = sum_k vf)
  pull       = (1/max(n,1)) * sum_m (1/max(cnt,1)) * sum_k vf * mean_t (g-mean)^2
  push       = 0.5/max(n(n-1),1) * sum_{i!=j valid} exp(-||mean_i-mean_j||^2)  (if n>1)
  out[b]     = [push, pull]

Sharding: batch dim B=64 split across 8 cores (8 images each); all reductions
are batch-local, no collectives; host concatenates per-core outputs.

Gather strategy: instead of 34 per-joint indirect DMAs (~1us of Pool SWDGE
descriptor-generation each), use TWO InstDMAGatherAnt instructions, one per
4-image quad. Each consumes int16 *block* indices and fetches the 256-byte
block (8 tag rows) containing each joint's row:
  - per-core tags are viewed as two [32768, 64] f32 halves (4 images each) so
    block indices fit int16's positive range,
  - item (slot k, partition p) of a gather lands at out[p, k, 0:64]; we place
    persons on partitions (120 of 128 used) and joints on slots,
  - the 8->1 sub-row selection is done on-chip with host-built masks
    M[p,k,s] = vf * (row & 7 == s), broadcast over the tag dim via a
    stride-0 AP, fused into one tensor_tensor_reduce per quad (gm = blk * M),
  - per-person sums then never need per-joint tensors: S1 = sum_{k,s} gm
    (DVE reduce keeping t), S2 = sum gm*blk = sum vf*g^2 (second ttr).
The remaining tail matches the old kernel: mean -> PE transpose -> Gram ->
exp(2G - r_i) with row-norm bias -> masked matmuls for push, and
pull = (S2 + cnt*rneg)/(T*cnt); n-derived factors come from the masks alone
and complete during the gather window.
"""

import numpy as np

import concourse.bass as bass
import concourse.tile as tile
from concourse import bacc, mybir
from concourse.bass_utils import run_bass_kernel_spmd
from concourse.tile_rust import add_dep_helper

B, N, T = 64, 65536, 8
M, K = 30, 17
NCORES = 8
BL = B // NCORES   # images per core
TB = 4             # images per quad
NQ = BL // TB      # quads per core (2)
P = 128            # partitions (TB*M = 120 used)
PU = TB * M        # used partitions
NI = K * P         # num_idxs per quad gather (2176)
NBLK = TB * N // 8  # 32768 blocks per tags half
F32 = mybir.dt.float32
F32R = mybir.dt.float32r
I16 = mybir.dt.int16
U8 = mybir.dt.uint8

ALU = mybir.AluOpType
AX = mybir.AxisListType
ACT = mybir.ActivationFunctionType

# packed-constant byte layout (per partition) for the two input DMAs
#   pk1: [idxA (272B) | idxB (272B)]                      -> needed first
#   pk2: [M_A (544B) | M_B (544B) | ident (512B) | bmv(16B) | ones(4B) |
#         cnt(8B) | icnt(8B) | wt(32B) | wtn(32B) | fac(64B, partition 0)]
PK1_B = 544
PK2_B = 544 + 544 + 512 + 16 + 4 + 8 + 8 + 32 + 32 + 64


def build_nc():
    nc = bacc.Bacc("TRN2", target_bir_lowering=False, debug=False, num_devices=NCORES)
    tags_a = nc.declare_dram_parameter("tags_a", [NBLK, 64], F32, isOutput=False)
    tags_b = nc.declare_dram_parameter("tags_b", [NBLK, 64], F32, isOutput=False)
    pk1_ext = nc.declare_dram_parameter("pk1", [P, PK1_B], U8, isOutput=False)
    pk2_ext = nc.declare_dram_parameter("pk2", [P, PK2_B], U8, isOutput=False)
    out_ext = nc.declare_dram_parameter("out", [1, BL * 2], F32, isOutput=True)

    with tile.TileContext(nc) as tc:
        with (
            tc.tile_pool(name="sb", bufs=1) as sb,
            tc.tile_pool(name="ps", bufs=1, space="PSUM") as ps,
        ):
            pk1 = sb.tile([P, PK1_B], U8, tag="pk1")
            nc.sync.dma_start(pk1[:], pk1_ext[:])
            pk2 = sb.tile([P, PK2_B], U8, tag="pk2")
            nc.scalar.dma_start(pk2[:], pk2_ext[:])

            idx = [pk1[:, 0:272].bitcast(I16), pk1[:, 272:544].bitcast(I16)]
            msk = [
                pk2[:, 0:544].bitcast(F32).rearrange("p (k s) -> p k s", s=8),
                pk2[:, 544:1088].bitcast(F32).rearrange("p (k s) -> p k s", s=8),
            ]
            ident = pk2[:, 1088:1600].bitcast(F32)
            bmv = pk2[:, 1600:1616].bitcast(F32)
            ones_c = pk2[:, 1616:1620].bitcast(F32)
            cnt_q = [pk2[:, 1620 + 4 * q:1624 + 4 * q].bitcast(F32) for q in range(NQ)]
            icnt_q = [pk2[:, 1628 + 4 * q:1632 + 4 * q].bitcast(F32) for q in range(NQ)]
            wt_q = [pk2[:, 1636 + 16 * q:1652 + 16 * q].bitcast(F32) for q in range(NQ)]
            wtn_q = [pk2[:, 1668 + 16 * q:1684 + 16 * q].bitcast(F32) for q in range(NQ)]
            fac = pk2[0:1, 1700:1764].bitcast(F32)

            srcs = [tags_a, tags_b]
            blk = [sb.tile([P, K, 64], F32, tag=f"blk{q}", name=f"blk{q}") for q in range(NQ)]
            gm = [sb.tile([P, K, 8, 8], F32, tag=f"gm{q}", name=f"gm{q}") for q in range(NQ)]
            sq = sb.tile([P, K * 64], F32, tag="sq")
            stats_ps = ps.tile([1, NQ * 8], F32, tag="stats", bufs=1, space="PSUM")
            out_sb = sb.tile([1, BL * 2], F32, tag="out_sb")

            # quad gathers, chunked: HW caps one dma_gather at 1024 indices,
            # so each quad is 3 instructions (slots 0-7 / 8-15 / 16).
            # Order A1 A2 B1 A3 B2 B3: quad A completes early (its whole tail
            # hides under quad B's transfers) and the bus stays near-saturated.
            CHUNKS = [(0, 0, 8), (0, 8, 16), (1, 0, 8), (0, 16, 17), (1, 8, 16), (1, 16, 17)]
            CHUNK_OFF = {(0, 8): 0, (8, 16): 128, (16, 17): 256}
            for (q, k0, k1) in CHUNKS:
                ob = 272 * q + CHUNK_OFF[(k0, k1)]
                ni = (k1 - k0) * P
                nc.gpsimd.dma_gather(
                    out_ap=blk[q][:, k0:k1, :],
                    in_ap=srcs[q][:],
                    idxs_ap=pk1[:, ob:ob + ni // 8].bitcast(I16),
                    num_idxs=ni,
                    num_idxs_reg=ni,
                    elem_size=64,
                )

            # per-quad pipeline, processed per gather chunk in arrival order so
            # the DVE/ACT streams never head-of-line block on late data
            s2p = sb.tile([P, NQ * 3], F32, tag="s2p")
            s2 = sb.tile([P, NQ], F32, tag="s2")
            s1p = [sb.tile([P, 24], F32, tag=f"s1p{q}", name=f"s1p{q}") for q in range(NQ)]
            s1 = [sb.tile([P, 8], F32, tag=f"s1{q}", name=f"s1{q}") for q in range(NQ)]
            mn = [sb.tile([P, 8], F32, tag=f"mn{q}", name=f"mn{q}") for q in range(NQ)]

            def chunk_ops(q, k0, k1, j):
                nk = k1 - k0
                gm_s = gm[q][:, k0:k1]
                nc.vector.tensor_tensor(
                    out=gm_s,
                    in0=blk[q][:, k0:k1, :].rearrange("p k (s t) -> p k s t", t=8),
                    in1=msk[q][:, k0:k1].broadcast_to((P, nk, 8, 8)),
                    op=ALU.mult,
                )
                nc.vector.reduce_sum(
                    out=s1p[q][:, 8 * j:8 * (j + 1)],
                    in_=gm_s.rearrange("p k s t -> p t (k s)"),
                    axis=AX.X,
                )
                nc.scalar.activation(
                    out=sq[:, k0 * 64:k1 * 64],
                    in_=gm_s.rearrange("p k s t -> p (k s t)"),
                    func=ACT.Square, accum_out=s2p[:, 3 * q + j:3 * q + j + 1],
                )

            def finish_core(q):
                sp = s1p[q]
                nc.vector.tensor_tensor(out=s1[q][:], in0=sp[:, 0:8], in1=sp[:, 8:16], op=ALU.add)
                nc.vector.tensor_tensor(out=s1[q][:], in0=s1[q][:], in1=sp[:, 16:24], op=ALU.add)
                nc.vector.tensor_scalar_mul(out=mn[q][:], in0=s1[q][:], scalar1=icnt_q[q])

                # rneg = -||mean||^2 ; er = exp(rneg)
                msq = sb.tile([P, 8], F32, tag=f"msq{q}", name=f"msq{q}")
                nc.vector.scalar_tensor_tensor(
                    out=msq[:], in0=mn[q][:], scalar=-1.0, in1=mn[q][:],
                    op0=ALU.mult, op1=ALU.mult,
                )
                rneg[q] = sb.tile([P, 1], F32, tag=f"rneg{q}", name=f"rneg{q}")
                nc.vector.reduce_sum(out=rneg[q][:], in_=msq[:], axis=AX.X)
                er[q] = sb.tile([P, 1], F32, tag=f"er{q}", name=f"er{q}")
                nc.scalar.activation(out=er[q][:], in_=rneg[q][:], func=ACT.Exp)

                # meanT via PE transpose (copy to SBUF on ACT), Gram = meanT.T meanT
                # fp32r halves/quarters the PE row cost at equal bit-width
                tp = ps.tile([8, P], F32, tag="psA", bufs=2, space="PSUM", name=f"tp{q}")
                nc.tensor.matmul(out=tp[:], lhsT=mn[q][:], rhs=ident,
                                 is_transpose=True, start=True, stop=True)
                meant = sb.tile([8, P], F32, tag=f"meant{q}", name=f"meant{q}")
                nc.scalar.copy(out=meant[:], in_=tp[:])
                d2p = ps.tile([P, P], F32, tag="psB", bufs=2, space="PSUM", name=f"d2p{q}")
                nc.tensor.matmul(out=d2p[:], lhsT=meant[:], rhs=meant[:],
                                 start=True, stop=True)

                # ee[i,j] = exp(2G[i,j] - r_i); exp(-r_j) folded into wt below
                ee[q] = sb.tile([P, P], F32, tag=f"ee{q}", name=f"ee{q}")
                nc.scalar.activation(out=ee[q][:], in_=d2p[:], func=ACT.Exp, scale=2.0, bias=rneg[q][:])

            def finish_stats(q):
                nc.vector.reduce_sum(
                    out=s2[:, q:q + 1], in_=s2p[:, 3 * q:3 * q + 3], axis=AX.X,
                )
                # pull: pp = (S2 + cnt*rneg) * icnt / T
                crn = sb.tile([P, 1], F32, tag=f"crn{q}", name=f"crn{q}")
                nc.vector.tensor_tensor(out=crn[:], in0=cnt_q[q], in1=rneg[q][:], op=ALU.mult)
                nc.vector.tensor_tensor(out=crn[:], in0=s2[:, q:q + 1], in1=crn[:], op=ALU.add)
                pp = sb.tile([P, 1], F32, tag=f"pp{q}", name=f"pp{q}")
                nc.vector.tensor_scalar(
                    out=pp[:], in0=crn[:], scalar1=icnt_q[q], scalar2=1.0 / T,
                    op0=ALU.mult, op1=ALU.mult,
                )

                wtq = wt_q[q]
                srhs = sb.tile([P, 12], F32, tag=f"srhs{q}", name=f"srhs{q}")
                nc.vector.tensor_scalar_mul(out=srhs[:, 8:12], in0=wtq, scalar1=er[q][:])
                up = ps.tile([P, TB], F32, tag="psC", bufs=2, space="PSUM", name=f"up{q}")
                nc.tensor.matmul(out=up[:], lhsT=ee[q][:], rhs=wtq,
                                 start=True, stop=True)
                nc.vector.tensor_tensor(out=srhs[:, 0:4], in0=srhs[:, 8:12], in1=up[:], op=ALU.mult)
                nc.vector.tensor_scalar_mul(out=srhs[:, 4:8], in0=bmv, scalar1=pp[:])

                # per-image sums; the -n correction accumulates into the s cols
                nc.tensor.matmul(
                    out=stats_ps[0:1, 8 * q + 4:8 * (q + 1)], lhsT=ones_c, rhs=srhs[:, 4:8],
                    start=True, stop=True,
                )
                nc.tensor.matmul(
                    out=stats_ps[0:1, 8 * q:8 * q + 4], lhsT=ones_c, rhs=wtn_q[q],
                    start=True, stop=False,
                )
                nc.tensor.matmul(
                    out=stats_ps[0:1, 8 * q:8 * q + 4], lhsT=ones_c, rhs=srhs[:, 0:4],
                    start=False, stop=True,
                )

            rneg = [None, None]
            er = [None, None]
            ee = [None, None]
            jn = {0: 0, 1: 0}
            for (q, k0, k1) in CHUNKS:
                chunk_ops(q, k0, k1, jn[q])
                jn[q] += 1
                if jn[q] == 3:
                    finish_core(q)
            finish_stats(0)
            finish_stats(1)

            # final: one op applies both factors, one DMA stores [1, 16]
            sv = stats_ps[:].rearrange("p (q c b) -> p c q b", q=NQ, c=2, b=TB)
            ov = out_sb[:].rearrange("p (q b c) -> p c q b", q=NQ, b=TB, c=2)
            fv = fac.rearrange("p (c q b) -> p c q b", c=2, q=NQ, b=TB)
            nc.vector.tensor_tensor(out=ov, in0=sv, in1=fv, op=ALU.mult)
            nc.sync.dma_start(out_ext[:], out_sb[:])

    nc.compile()
    return nc


_NC_CACHE = {}


def _get_nc():
    if "nc" not in _NC_CACHE:
        _NC_CACHE["nc"] = build_nc()
    return _NC_CACHE["nc"]


def _pack_consts(idx16, msks, vfq):
    """idx16: [NQ,128,136] i16 wrapped; msks: [NQ,128,K,8] f32; vfq: [NQ,TB,M,K]."""
    pk1 = np.zeros((P, PK1_B), dtype=np.uint8)
    pk1[:, 0:272] = idx16[0].view(np.uint8).reshape(P, 272)
    pk1[:, 272:544] = idx16[1].view(np.uint8).reshape(P, 272)
    pk2 = np.zeros((P, PK2_B), dtype=np.uint8)
    pk2[:, 0:544] = np.ascontiguousarray(msks[0]).view(np.uint8).reshape(P, 544)
    pk2[:, 544:1088] = np.ascontiguousarray(msks[1]).view(np.uint8).reshape(P, 544)
    ident = np.eye(P, dtype=np.float32)
    pk2[:, 1088:1600] = ident.view(np.uint8).reshape(P, 512)
    bmv = np.zeros((P, TB), dtype=np.float32)
    for b in range(TB):
        bmv[b * M:(b + 1) * M, b] = 1.0
    pk2[:, 1600:1616] = bmv.view(np.uint8).reshape(P, 16)
    ones = np.ones((P, 1), dtype=np.float32)
    pk2[:, 1616:1620] = ones.view(np.uint8).reshape(P, 4)
    # per-person / per-image normalizers (keypoint metadata only)
    cnt = np.zeros((P, NQ), dtype=np.float32)
    fac = np.zeros(2 * BL, dtype=np.float32)
    wt = np.zeros((P, NQ * TB), dtype=np.float32)
    for q in range(NQ):
        cq = vfq[q].sum(axis=2).reshape(PU)          # [120]
        cnt[:PU, q] = cq
        h = np.minimum(cq, 1.0)
        wt[:PU, TB * q:TB * (q + 1)] = bmv[:PU] * h[:, None]
        n = h.reshape(TB, M).sum(axis=1)             # [4]
        iq = 0.5 * np.clip(n - 1.0, 0.0, 1.0) / np.maximum(n * (n - 1.0), 1.0)
        ipn = 1.0 / np.maximum(n, 1.0)
        fac[0 * BL + TB * q:0 * BL + TB * (q + 1)] = iq
        fac[1 * BL + TB * q:1 * BL + TB * (q + 1)] = ipn
    icnt = 1.0 / np.maximum(cnt, 1.0)
    pk2[:, 1620:1628] = cnt.view(np.uint8).reshape(P, 8)
    pk2[:, 1628:1636] = icnt.view(np.uint8).reshape(P, 8)
    pk2[:, 1636:1668] = wt.view(np.uint8).reshape(P, 32)
    pk2[:, 1668:1700] = (-wt).view(np.uint8).reshape(P, 32)
    pk2[0, 1700:1764] = fac.view(np.uint8)
    return pk1, pk2


def make_in_maps(tags, keypoints):
    tags = np.asarray(tags, dtype=np.float32)
    kp = np.asarray(keypoints)
    idx = np.clip(kp[..., 0].astype(np.int64), 0, N - 1)   # [B, M, K]
    vf = (kp[..., 1] > 0).astype(np.float32)               # [B, M, K]

    in_maps = []
    for c in range(NCORES):
        halves = []
        idx16 = np.zeros((NQ, P, 136), dtype=np.int16)
        msks = np.zeros((NQ, P, K, 8), dtype=np.float32)
        vfq = np.zeros((NQ, TB, M, K), dtype=np.float32)
        for q in range(NQ):
            sl = slice(BL * c + TB * q, BL * c + TB * (q + 1))
            halves.append(np.ascontiguousarray(tags[sl].reshape(NBLK, 64)))
            iq_ = idx[sl]   # [TB, M, K]
            vq = vf[sl]
            vfq[q] = vq
            # flat row within half -> block and sub-row
            rows = (np.arange(TB, dtype=np.int64)[:, None, None] * N + iq_)  # [TB, M, K]
            blk_q = (rows >> 3).astype(np.int16)      # [TB, M, K] in [0, 32768)
            sub_q = (rows & 7).astype(np.int64)
            # item (slot k, partition p): p = img*M + person; wrapped idx
            # layout per gather chunk (slots 0-7 / 8-15 / 16)
            pidx = np.arange(PU)
            img, per = pidx // M, pidx % M
            col = 0
            for (k0, k1) in ((0, 8), (8, 16), (16, 17)):
                ni = (k1 - k0) * P
                vals = np.zeros(ni, dtype=np.int16)
                for k in range(k0, k1):
                    vals[(k - k0) * P + pidx] = blk_q[img, per, k]
                wrapped = vals.reshape(ni // 16, 16).T   # [16, ni/16]
                idx16[q][:, col:col + ni // 16] = np.tile(wrapped, (8, 1))
                col += ni // 16
            # masks
            mq = np.zeros((P, K, 8), dtype=np.float32)
            mq[pidx[:, None], np.arange(K)[None, :], sub_q[img, per, :]] = vq[img, per, :]
            msks[q] = mq
        pk1, pk2 = _pack_consts(idx16, msks, vfq)
        in_maps.append({
            "tags_a": halves[0],
            "tags_b": halves[1],
            "pk1": pk1,
            "pk2": pk2,
        })
    return in_maps


def kernel(tags, keypoints):
    nc = _get_nc()
    in_maps = make_in_maps(tags, keypoints)
    last_err = None
    for _attempt in range(3):
        try:
            res = run_bass_kernel_spmd(nc, in_maps, core_ids=list(range(NCORES))).results
            break
        except Exception as e:  # a crashed predecessor can leave the NC wedged;
            last_err = e        # the failed attempt clears it, so retry
            import time
            time.sleep(1.0)
    else:
        raise last_err
    out = np.concatenate([res[c]["out"].reshape(BL, 2) for c in range(NCORES)], axis=0)
    return out.astype(np.float32)


# revision 15
# speedup vs baseline: 2.1910x; 1.0024x over previous
"""Associative-embedding (AE) loss on 8 TRN2 NeuronCores, data-parallel over batch.

Reference computation (per batch image b):
  g[m,k,:]   = tags[b, idx[b,m,k], :]                       (gather, T=8)
  mean[m,:]  = sum_k vf*g / max(cnt,1)                      (cnt = sum_k vf)
  pull       = (1/max(n,1)) * sum_m (1/max(cnt,1)) * sum_k vf * mean_t (g-mean)^2
  push       = 0.5/max(n(n-1),1) * sum_{i!=j valid} exp(-||mean_i-mean_j||^2)  (if n>1)
  out[b]     = [push, pull]

Sharding: batch dim B=64 split across 8 cores (8 images each); all reductions
are batch-local, no collectives; host concatenates per-core outputs.

Gather strategy: instead of 34 per-joint indirect DMAs (~1us of Pool SWDGE
descriptor-generation each), use TWO InstDMAGatherAnt instructions, one per
4-image quad. Each consumes int16 *block* indices and fetches the 256-byte
block (8 tag rows) containing each joint's row:
  - per-core tags are viewed as two [32768, 64] f32 halves (4 images each) so
    block indices fit int16's positive range,
  - item (slot k, partition p) of a gather lands at out[p, k, 0:64]; we place
    persons on partitions (120 of 128 used) and joints on slots,
  - the 8->1 sub-row selection is done on-chip with host-built masks
    M[p,k,s] = vf * (row & 7 == s), broadcast over the tag dim via a
    stride-0 AP, fused into one tensor_tensor_reduce per quad (gm = blk * M),
  - per-person sums then never need per-joint tensors: S1 = sum_{k,s} gm
    (DVE reduce keeping t), S2 = sum gm*blk = sum vf*g^2 (second ttr).
The remaining tail matches the old kernel: mean -> PE transpose -> Gram ->
exp(2G - r_i) with row-norm bias -> masked matmuls for push, and
pull = (S2 + cnt*rneg)/(T*cnt); n-derived factors come from the masks alone
and complete during the gather window.
"""

import numpy as np

import concourse.bass as bass
import concourse.tile as tile
from concourse import bacc, mybir
from concourse.bass_utils import run_bass_kernel_spmd
from concourse.tile_rust import add_dep_helper

B, N, T = 64, 65536, 8
M, K = 30, 17
NCORES = 8
BL = B // NCORES   # images per core
TB = 4             # images per quad
NQ = BL // TB      # quads per core (2)
P = 128            # partitions (TB*M = 120 used)
PU = TB * M        # used partitions
NI = K * P         # num_idxs per quad gather (2176)
NBLK = TB * N // 8  # 32768 blocks per tags half
F32 = mybir.dt.float32
F32R = mybir.dt.float32r
I16 = mybir.dt.int16
U8 = mybir.dt.uint8

ALU = mybir.AluOpType
AX = mybir.AxisListType
ACT = mybir.ActivationFunctionType

# packed-constant byte layout (per partition) for the two input DMAs
#   pk1: [idxA (272B) | idxB (272B)]                      -> needed first
#   pk2: [M_A (544B) | M_B (544B) | ident (512B) | bmv(16B) | ones(4B) |
#         cnt(8B) | icnt(8B) | wt(32B) | wtn(32B) | fac(64B, partition 0)]
PK1_B = 544
PK2_B = 544 + 544 + 512 + 16 + 4 + 8 + 8 + 32 + 32 + 64


def build_nc():
    nc = bacc.Bacc("TRN2", target_bir_lowering=False, debug=False, num_devices=NCORES)
    tags_a = nc.declare_dram_parameter("tags_a", [NBLK, 64], F32, isOutput=False)
    tags_b = nc.declare_dram_parameter("tags_b", [NBLK, 64], F32, isOutput=False)
    pk1_ext = nc.declare_dram_parameter("pk1", [P, PK1_B], U8, isOutput=False)
    pk2_ext = nc.declare_dram_parameter("pk2", [P, PK2_B], U8, isOutput=False)
    out_ext = nc.declare_dram_parameter("out", [1, BL * 2], F32, isOutput=True)

    with tile.TileContext(nc) as tc:
        with (
            tc.tile_pool(name="sb", bufs=1) as sb,
            tc.tile_pool(name="ps", bufs=1, space="PSUM") as ps,
        ):
            pk1 = sb.tile([P, PK1_B], U8, tag="pk1")
            nc.sync.dma_start(pk1[:], pk1_ext[:])
            pk2 = sb.tile([P, PK2_B], U8, tag="pk2")
            nc.scalar.dma_start(pk2[:], pk2_ext[:])

            idx = [pk1[:, 0:272].bitcast(I16), pk1[:, 272:544].bitcast(I16)]
            msk = [
                pk2[:, 0:544].bitcast(F32).rearrange("p (k s) -> p k s", s=8),
                pk2[:, 544:1088].bitcast(F32).rearrange("p (k s) -> p k s", s=8),
            ]
            ident = pk2[:, 1088:1600].bitcast(F32)
            bmv = pk2[:, 1600:1616].bitcast(F32)
            ones_c = pk2[:, 1616:1620].bitcast(F32)
            cnt_q = [pk2[:, 1620 + 4 * q:1624 + 4 * q].bitcast(F32) for q in range(NQ)]
            icnt_q = [pk2[:, 1628 + 4 * q:1632 + 4 * q].bitcast(F32) for q in range(NQ)]
            wt_q = [pk2[:, 1636 + 16 * q:1652 + 16 * q].bitcast(F32) for q in range(NQ)]
            wtn_q = [pk2[:, 1668 + 16 * q:1684 + 16 * q].bitcast(F32) for q in range(NQ)]
            fac = pk2[0:1, 1700:1764].bitcast(F32)

            srcs = [tags_a, tags_b]
            blk = [sb.tile([P, K, 64], F32, tag=f"blk{q}", name=f"blk{q}") for q in range(NQ)]
            gm = [sb.tile([P, K, 8, 8], F32, tag=f"gm{q}", name=f"gm{q}") for q in range(NQ)]
            sq = sb.tile([P, K * 64], F32, tag="sq")
            stats_ps = ps.tile([1, NQ * 8], F32, tag="stats", bufs=1, space="PSUM")
            out_sb = sb.tile([1, BL * 2], F32, tag="out_sb")

            # quad gathers, chunked: HW caps one dma_gather at 1024 indices,
            # so each quad is 3 instructions (slots 0-7 / 8-15 / 16).
            # Order A1 A2 B1 A3 B2 B3: quad A completes early (its whole tail
            # hides under quad B's transfers) and the bus stays near-saturated.
            CHUNKS = [(0, 0, 8), (0, 8, 16), (1, 0, 8), (0, 16, 17), (1, 8, 16), (1, 16, 17)]
            CHUNK_OFF = {(0, 8): 0, (8, 16): 128, (16, 17): 256}
            for (q, k0, k1) in CHUNKS:
                ob = 272 * q + CHUNK_OFF[(k0, k1)]
                ni = (k1 - k0) * P
                nc.gpsimd.dma_gather(
                    out_ap=blk[q][:, k0:k1, :],
                    in_ap=srcs[q][:],
                    idxs_ap=pk1[:, ob:ob + ni // 8].bitcast(I16),
                    num_idxs=ni,
                    num_idxs_reg=ni,
                    elem_size=64,
                )

            # per-quad pipeline, processed per gather chunk in arrival order so
            # the DVE/ACT streams never head-of-line block on late data
            s2p = sb.tile([P, NQ * 3], F32, tag="s2p")
            s2 = sb.tile([P, NQ], F32, tag="s2")
            s1p = [sb.tile([P, 24], F32, tag=f"s1p{q}", name=f"s1p{q}") for q in range(NQ)]
            s1 = [sb.tile([P, 8], F32, tag=f"s1{q}", name=f"s1{q}") for q in range(NQ)]
            mn = [sb.tile([P, 8], F32, tag=f"mn{q}", name=f"mn{q}") for q in range(NQ)]

            def chunk_ops(q, k0, k1, j):
                nk = k1 - k0
                gm_s = gm[q][:, k0:k1]
                nc.vector.tensor_tensor(
                    out=gm_s,
                    in0=blk[q][:, k0:k1, :].rearrange("p k (s t) -> p k s t", t=8),
                    in1=msk[q][:, k0:k1].broadcast_to((P, nk, 8, 8)),
                    op=ALU.mult,
                )
                nc.vector.reduce_sum(
                    out=s1p[q][:, 8 * j:8 * (j + 1)],
                    in_=gm_s.rearrange("p k s t -> p t (k s)"),
                    axis=AX.X,
                )
                nc.scalar.activation(
                    out=sq[:, k0 * 64:k1 * 64],
                    in_=gm_s.rearrange("p k s t -> p (k s t)"),
                    func=ACT.Square, accum_out=s2p[:, 3 * q + j:3 * q + j + 1],
                )

            def finish_core(q):
                sp = s1p[q]
                nc.vector.tensor_tensor(out=s1[q][:], in0=sp[:, 0:8], in1=sp[:, 8:16], op=ALU.add)
                nc.vector.tensor_tensor(out=s1[q][:], in0=s1[q][:], in1=sp[:, 16:24], op=ALU.add)
                nc.vector.tensor_scalar_mul(out=mn[q][:], in0=s1[q][:], scalar1=icnt_q[q])

                # meanT via PE transpose, emitted right after mean so the PE
                # wait (coalesced to "all prior DVE ops done") fires earliest
                tp = ps.tile([8, P], F32, tag="psA", bufs=2, space="PSUM", name=f"tp{q}")
                nc.tensor.matmul(out=tp[:], lhsT=mn[q][:], rhs=ident,
                                 is_transpose=True, start=True, stop=True)

                # rneg = -||mean||^2 ; er = exp(rneg)
                msq = sb.tile([P, 8], F32, tag=f"msq{q}", name=f"msq{q}")
                nc.vector.scalar_tensor_tensor(
                    out=msq[:], in0=mn[q][:], scalar=-1.0, in1=mn[q][:],
                    op0=ALU.mult, op1=ALU.mult,
                )
                rneg[q] = sb.tile([P, 1], F32, tag=f"rneg{q}", name=f"rneg{q}")
                nc.vector.reduce_sum(out=rneg[q][:], in_=msq[:], axis=AX.X)
                meant = sb.tile([8, P], F32, tag=f"meant{q}", name=f"meant{q}")
                nc.vector.tensor_copy(out=meant[:], in_=tp[:])
                er[q] = sb.tile([P, 1], F32, tag=f"er{q}", name=f"er{q}")
                nc.scalar.activation(out=er[q][:], in_=rneg[q][:], func=ACT.Exp)

                d2p = ps.tile([P, P], F32, tag="psB", bufs=2, space="PSUM", name=f"d2p{q}")
                nc.tensor.matmul(out=d2p[:], lhsT=meant[:], rhs=meant[:],
                                 start=True, stop=True)

                # ee[i,j] = exp(2G[i,j] - r_i); exp(-r_j) folded into wt below
                ee[q] = sb.tile([P, P], F32, tag=f"ee{q}", name=f"ee{q}")
                nc.scalar.activation(out=ee[q][:], in_=d2p[:], func=ACT.Exp, scale=2.0, bias=rneg[q][:])

            def finish_stats(q):
                nc.vector.reduce_sum(
                    out=s2[:, q:q + 1], in_=s2p[:, 3 * q:3 * q + 3], axis=AX.X,
                )
                # pull: pp = (S2 + cnt*rneg) * icnt / T
                crn = sb.tile([P, 1], F32, tag=f"crn{q}", name=f"crn{q}")
                nc.vector.tensor_tensor(out=crn[:], in0=cnt_q[q], in1=rneg[q][:], op=ALU.mult)
                nc.vector.tensor_tensor(out=crn[:], in0=s2[:, q:q + 1], in1=crn[:], op=ALU.add)
                pp = sb.tile([P, 1], F32, tag=f"pp{q}", name=f"pp{q}")
                nc.vector.tensor_scalar(
                    out=pp[:], in0=crn[:], scalar1=icnt_q[q], scalar2=1.0 / T,
                    op0=ALU.mult, op1=ALU.mult,
                )

                wtq = wt_q[q]
                srhs = sb.tile([P, 12], F32, tag=f"srhs{q}", name=f"srhs{q}")
                nc.vector.tensor_scalar_mul(out=srhs[:, 8:12], in0=wtq, scalar1=er[q][:])
                up = ps.tile([P, TB], F32, tag="psC", bufs=2, space="PSUM", name=f"up{q}")
                nc.tensor.matmul(out=up[:], lhsT=ee[q][:], rhs=wtq,
                                 start=True, stop=True)
                nc.vector.tensor_tensor(out=srhs[:, 0:4], in0=srhs[:, 8:12], in1=up[:], op=ALU.mult)
                nc.vector.tensor_scalar_mul(out=srhs[:, 4:8], in0=bmv, scalar1=pp[:])

                # per-image sums; the -n correction accumulates into the s cols
                nc.tensor.matmul(
                    out=stats_ps[0:1, 8 * q + 4:8 * (q + 1)], lhsT=ones_c, rhs=srhs[:, 4:8],
                    start=True, stop=True,
                )
                nc.tensor.matmul(
                    out=stats_ps[0:1, 8 * q:8 * q + 4], lhsT=ones_c, rhs=wtn_q[q],
                    start=True, stop=False,
                )
                nc.tensor.matmul(
                    out=stats_ps[0:1, 8 * q:8 * q + 4], lhsT=ones_c, rhs=srhs[:, 0:4],
                    start=False, stop=True,
                )

            rneg = [None, None]
            er = [None, None]
            ee = [None, None]
            jn = {0: 0, 1: 0}
            for (q, k0, k1) in CHUNKS:
                chunk_ops(q, k0, k1, jn[q])
                jn[q] += 1
                if jn[q] == 3:
                    finish_core(q)
            finish_stats(0)
            finish_stats(1)

            # final: one op applies both factors, one DMA stores [1, 16]
            sv = stats_ps[:].rearrange("p (q c b) -> p c q b", q=NQ, c=2, b=TB)
            ov = out_sb[:].rearrange("p (q b c) -> p c q b", q=NQ, b=TB, c=2)
            fv = fac.rearrange("p (c q b) -> p c q b", c=2, q=NQ, b=TB)
            nc.vector.tensor_tensor(out=ov, in0=sv, in1=fv, op=ALU.mult)
            nc.sync.dma_start(out_ext[:], out_sb[:])

    nc.compile()
    return nc


_NC_CACHE = {}


def _get_nc():
    if "nc" not in _NC_CACHE:
        _NC_CACHE["nc"] = build_nc()
    return _NC_CACHE["nc"]


def _pack_consts(idx16, msks, vfq):
    """idx16: [NQ,128,136] i16 wrapped; msks: [NQ,128,K,8] f32; vfq: [NQ,TB,M,K]."""
    pk1 = np.zeros((P, PK1_B), dtype=np.uint8)
    pk1[:, 0:272] = idx16[0].view(np.uint8).reshape(P, 272)
    pk1[:, 272:544] = idx16[1].view(np.uint8).reshape(P, 272)
    pk2 = np.zeros((P, PK2_B), dtype=np.uint8)
    pk2[:, 0:544] = np.ascontiguousarray(msks[0]).view(np.uint8).reshape(P, 544)
    pk2[:, 544:1088] = np.ascontiguousarray(msks[1]).view(np.uint8).reshape(P, 544)
    ident = np.eye(P, dtype=np.float32)
    pk2[:, 1088:1600] = ident.view(np.uint8).reshape(P, 512)
    bmv = np.zeros((P, TB), dtype=np.float32)
    for b in range(TB):
        bmv[b * M:(b + 1) * M, b] = 1.0
    pk2[:, 1600:1616] = bmv.view(np.uint8).reshape(P, 16)
    ones = np.ones((P, 1), dtype=np.float32)
    pk2[:, 1616:1620] = ones.view(np.uint8).reshape(P, 4)
    # per-person / per-image normalizers (keypoint metadata only)
    cnt = np.zeros((P, NQ), dtype=np.float32)
    fac = np.zeros(2 * BL, dtype=np.float32)
    wt = np.zeros((P, NQ * TB), dtype=np.float32)
    for q in range(NQ):
        cq = vfq[q].sum(axis=2).reshape(PU)          # [120]
        cnt[:PU, q] = cq
        h = np.minimum(cq, 1.0)
        wt[:PU, TB * q:TB * (q + 1)] = bmv[:PU] * h[:, None]
        n = h.reshape(TB, M).sum(axis=1)             # [4]
        iq = 0.5 * np.clip(n - 1.0, 0.0, 1.0) / np.maximum(n * (n - 1.0), 1.0)
        ipn = 1.0 / np.maximum(n, 1.0)
        fac[0 * BL + TB * q:0 * BL + TB * (q + 1)] = iq
        fac[1 * BL + TB * q:1 * BL + TB * (q + 1)] = ipn
    icnt = 1.0 / np.maximum(cnt, 1.0)
    pk2[:, 1620:1628] = cnt.view(np.uint8).reshape(P, 8)
    pk2[:, 1628:1636] = icnt.view(np.uint8).reshape(P, 8)
    pk2[:, 1636:1668] = wt.view(np.uint8).reshape(P, 32)
    pk2[:, 1668:1700] = (-wt).view(np.uint8).reshape(P, 32)
    pk2[0, 1700:1764] = fac.view(np.uint8)
    return pk1, pk2


def make_in_maps(tags, keypoints):
    tags = np.asarray(tags, dtype=np.float32)
    kp = np.asarray(keypoints)
    idx = np.clip(kp[..., 0].astype(np.int64), 0, N - 1)   # [B, M, K]
    vf = (kp[..., 1] > 0).astype(np.float32)               # [B, M, K]

    in_maps = []
    for c in range(NCORES):
        halves = []
        idx16 = np.zeros((NQ, P, 136), dtype=np.int16)
        msks = np.zeros((NQ, P, K, 8), dtype=np.float32)
        vfq = np.zeros((NQ, TB, M, K), dtype=np.float32)
        for q in range(NQ):
            sl = slice(BL * c + TB * q, BL * c + TB * (q + 1))
            halves.append(np.ascontiguousarray(tags[sl].reshape(NBLK, 64)))
            iq_ = idx[sl]   # [TB, M, K]
            vq = vf[sl]
            vfq[q] = vq
            # flat row within half -> block and sub-row
            rows = (np.arange(TB, dtype=np.int64)[:, None, None] * N + iq_)  # [TB, M, K]
            blk_q = (rows >> 3).astype(np.int16)      # [TB, M, K] in [0, 32768)
            sub_q = (rows & 7).astype(np.int64)
            # item (slot k, partition p): p = img*M + person; wrapped idx
            # layout per gather chunk (slots 0-7 / 8-15 / 16)
            pidx = np.arange(PU)
            img, per = pidx // M, pidx % M
            col = 0
            for (k0, k1) in ((0, 8), (8, 16), (16, 17)):
                ni = (k1 - k0) * P
                vals = np.zeros(ni, dtype=np.int16)
                for k in range(k0, k1):
                    vals[(k - k0) * P + pidx] = blk_q[img, per, k]
                wrapped = vals.reshape(ni // 16, 16).T   # [16, ni/16]
                idx16[q][:, col:col + ni // 16] = np.tile(wrapped, (8, 1))
                col += ni // 16
            # masks
            mq = np.zeros((P, K, 8), dtype=np.float32)
            mq[pidx[:, None], np.arange(K)[None, :], sub_q[img, per, :]] = vq[img, per, :]
            msks[q] = mq
        pk1, pk2 = _pack_consts(idx16, msks, vfq)
        in_maps.append({
            "tags_a": halves[0],
            "tags_b": halves[1],
            "pk1": pk1,
            "pk2": pk2,
        })
    return in_maps


def kernel(tags, keypoints):
    nc = _get_nc()
    in_maps = make_in_maps(tags, keypoints)
    last_err = None
    for _attempt in range(3):
        try:
            res = run_bass_kernel_spmd(nc, in_maps, core_ids=list(range(NCORES))).results
            break
        except Exception as e:  # a crashed predecessor can leave the NC wedged;
            last_err = e        # the failed attempt clears it, so retry
            import time
            time.sleep(1.0)
    else:
        raise last_err
    out = np.concatenate([res[c]["out"].reshape(BL, 2) for c in range(NCORES)], axis=0)
    return out.astype(np.float32)
